# revision 1
# baseline (speedup 1.0000x reference)
"""Causal multi-head attention with RoPE on 8 Trainium2 NeuronCores.

Problem: B=2, L=2048, D_MODEL=1024, N_HEADS=16, D_K=64, theta=10000.
Sharding: data parallel on batch (2) x tensor parallel on heads (4 groups of
4 heads) = 8 cores. Each core computes its 4 heads' attention plus a partial
output projection; partials are summed on the host (Megatron row-parallel).

Per-core device design (v2 — fully pipelined):
- All tensors enter the device pre-transposed/permuted by the host so every
  matmul contraction dim lands on SBUF partitions. Matmuls run in fp32r
  (TF32) at full PE rate; attention weights/V run in bf16.
- Q/K head dims are split into "top" (even dims) / "bot" (odd dims) blocks of
  32 rows per head, so interleaved RoPE becomes full-128-partition DVE ops and
  the scores matmul runs as K=32 row-groups, 4 heads concurrent in the PE
  array via tile_position.
- Scores are computed transposed (scoresT[kv, q]) into two 2-head PSUM strips;
  exp is one ACT instruction per strip with exact-causal column slicing; the
  softmax denominator comes from an appended ones-column on V (row 64 of the
  AV output). Normalization is a K=1 broadcast matmul + 1 DVE mul.
- One shared PSUM pool (2x 2-bank score strips + 4x 1-bank AV accumulators)
  is reused by the projections and the output projection, so Tile can overlap
  all phases; program order is chunk-major so attention(c) starts as soon as
  proj(c) lands and outproj(c) follows normalize(c).
"""
import numpy as np
from contextlib import ExitStack

import concourse.bacc as bacc
import concourse.bass as bass
import concourse.mybir as mybir
import concourse.tile as tile
from concourse._compat import with_exitstack
from concourse.bass_utils import run_bass_kernel_spmd

F32 = mybir.dt.float32
F32R = mybir.dt.float32r
BF16 = mybir.dt.bfloat16

B, L, DM, NH, DK = 2, 2048, 1024, 16, 64
HPC = 4              # heads per core
THETA = 10000.0
CH = 512             # q/l chunk
NT = L // 128        # 16 kv tiles
NCH = L // CH        # 4 chunks

_cache = {}


@with_exitstack
def _attn_kernel(ctx: ExitStack, tc: tile.TileContext, outs, ins):
    nc = tc.nc
    xt, wq, wv, wo = ins["xt"], ins["wq"], ins["wv"], ins["wo"]
    cs, sn = ins["cs"], ins["sn"]
    out = outs["out"]
    AF = mybir.ActivationFunctionType

    consts = ctx.enter_context(tc.tile_pool(name="consts", bufs=1))
    persist = ctx.enter_context(tc.tile_pool(name="persist", bufs=1))
    ps = ctx.enter_context(tc.tile_pool(name="ps", bufs=1, space="PSUM"))
    xtp = ctx.enter_context(tc.tile_pool(name="xtp", bufs=14))
    csp = ctx.enter_context(tc.tile_pool(name="csp", bufs=3))
    ropet = ctx.enter_context(tc.tile_pool(name="ropet", bufs=3))
    epool = ctx.enter_context(tc.tile_pool(name="epool", bufs=4))
    ipool = ctx.enter_context(tc.tile_pool(name="ipool", bufs=2))
    opool = ctx.enter_context(tc.tile_pool(name="opool", bufs=4))

    # ---- weights / constants ----
    # wq and the first xt chunk are interleaved per-d so the d=0..7
    # accumulation chain starts as soon as each pair lands
    wq_sb = []
    xt0 = []
    for d in range(8):
        t_wq = consts.tile([128, 512], F32R, tag=f"wq{d}")
        nc.sync.dma_start(t_wq, wq[128 * d:128 * d + 128, :].bitcast(F32R))
        wq_sb.append(t_wq)
        t_x0 = xtp.tile([128, CH], F32R, tag="xt")
        nc.sync.dma_start(t_x0, xt[128 * d:128 * d + 128, 0:CH].bitcast(F32R))
        xt0.append(t_x0)
    ones_f = consts.tile([1, 64], F32)
    nc.vector.memset(ones_f, 1.0)
    ones_sb = consts.tile([1, 64], F32R)
    nc.vector.tensor_copy(ones_sb, ones_f)
    # lower-triangular keep-mask (keep iff q_local >= kv_local)
    tri = consts.tile([128, 128], BF16)
    nc.vector.memset(tri, 1.0)
    nc.gpsimd.affine_select(tri, tri, pattern=[[1, 128]],
                            compare_op=mybir.AluOpType.is_ge, fill=0.0,
                            base=0, channel_multiplier=-1)

    # persistent activations
    qt_t = persist.tile([128, L], F32R)   # Q tops  (4h x 32)
    qt_b = persist.tile([128, L], F32R)   # Q bots
    kt_t = persist.tile([128, L], F32R)   # K tops
    kt_b = persist.tile([128, L], F32R)   # K bots
    v_sb = []
    for t in range(NT):
        t_v = persist.tile([128, HPC * 65], BF16, tag=f"v{t}")
        v_sb.append(t_v)
    ho = []
    for j in range(2):
        t_ho = persist.tile([128, L], F32R, tag=f"ho{j}")
        ho.append(t_ho)

    wv_sb = []
    wo_sb = []

    def _load_wv():
        res = []
        for d in range(8):
            t_wv = consts.tile([128, 256], F32R, tag=f"wv{d}")
            nc.sync.dma_start(t_wv, wv[128 * d:128 * d + 128, :].bitcast(F32R))
            res.append(t_wv)
        return res

    def _load_wo():
        res = []
        for j in range(2):
            t_wo = consts.tile([128, DM], F32R, tag=f"wo{j}")
            nc.sync.dma_start(t_wo, wo[128 * j:128 * j + 128, :].bitcast(F32R))
            res.append(t_wo)
        return res

    state = {}

    def emit_xt_dmas(c):
        lsl = slice(CH * c, CH * (c + 1))
        xt_c = []
        for d in range(8):
            t_x = xtp.tile([128, CH], F32R, tag="xt")
            nc.sync.dma_start(t_x, xt[128 * d:128 * d + 128, lsl].bitcast(F32R))
            xt_c.append(t_x)
        cs_c = csp.tile([128, CH], F32, tag="cs")
        nc.sync.dma_start(cs_c, cs[:, lsl])
        sn_c = csp.tile([128, CH], F32, tag="sn")
        nc.sync.dma_start(sn_c, sn[:, lsl])
        state[c] = (xt_c, cs_c, sn_c)

    def emit_qk_part(c, part):
        """QKV projection + RoPE for Q (part=0) or K (part=1) of chunk c."""
        xt_c, cs_c, sn_c = state[c]
        lsl = slice(CH * c, CH * (c + 1))
        dst_t, dst_b = (qt_t, qt_b) if part == 0 else (kt_t, kt_b)
        ps_t = ps.tile([128, CH], F32, tag="scP0")
        ps_b = ps.tile([128, CH], F32, tag="scP1")
        for d in range(8):
            esl_t = slice(256 * part, 256 * part + 128)
            esl_b = slice(256 * part + 128, 256 * part + 256)
            nc.tensor.matmul(ps_t, wq_sb[d][:, esl_t], xt_c[d][:],
                             start=(d == 0), stop=(d == 7))
            nc.tensor.matmul(ps_b, wq_sb[d][:, esl_b], xt_c[d][:],
                             start=(d == 0), stop=(d == 7))
        # rope: top' = top*c - bot*s ; bot' = bot*c + top*s
        t1 = ropet.tile([128, CH], F32, tag="t1")
        t2 = ropet.tile([128, CH], F32, tag="t2")
        nc.vector.tensor_mul(t1, ps_t, cs_c)
        nc.vector.tensor_mul(t2, ps_b, sn_c)
        nc.vector.tensor_sub(dst_t[:, lsl], t1, t2)
        t3 = ropet.tile([128, CH], F32, tag="t1")
        t4 = ropet.tile([128, CH], F32, tag="t2")
        nc.vector.tensor_mul(t3, ps_b, cs_c)
        nc.vector.tensor_mul(t4, ps_t, sn_c)
        nc.vector.tensor_add(dst_b[:, lsl], t3, t4)

    def emit_v_tiles(c):
        xt_c, _, _ = state[c]
        for t in range(4 * c, 4 * c + 4):
            ps_v = ps.tile([128, 256], F32, tag=f"av{t % 4}")
            lo = 128 * t - CH * c
            for d in range(8):
                nc.tensor.matmul(ps_v, xt_c[d][:, lo:lo + 128], wv_sb[d][:],
                                 start=(d == 0), stop=(d == 7))
            vdst = v_sb[t][:].rearrange("p (h x) -> p h x", x=65)[:, :, 0:64]
            vsrc = ps_v[:].rearrange("p (h x) -> p h x", x=64)
            nc.vector.tensor_copy(vdst, vsrc)
            nc.vector.memset(v_sb[t][:, 64:HPC * 65:65], 1.0)

    # software pipeline: proj(c+1) is emitted inside attention(c)
    cs_0 = csp.tile([128, CH], F32, tag="cs")
    nc.sync.dma_start(cs_0, cs[:, 0:CH])
    sn_0 = csp.tile([128, CH], F32, tag="sn")
    nc.sync.dma_start(sn_0, sn[:, 0:CH])
    state[0] = (xt0, cs_0, sn_0)
    emit_qk_part(0, 0)
    wv_sb.extend(_load_wv())
    emit_qk_part(0, 1)
    wo_sb.extend(_load_wo())
    emit_v_tiles(0)

    for c in range(NCH):
        if c + 1 < NCH:
            emit_xt_dmas(c + 1)
        # ---- attention for this chunk ----
        qsl = slice(CH * c, CH * (c + 1))
        av = []
        for h in range(HPC):
            t_av = ps.tile([65, CH], F32, tag=f"av{h}")
            av.append(t_av)
        ntile = 4 * c + 4
        for t in range(ntile):
            ksl = slice(128 * t, 128 * t + 128)
            diag = (t // 4 == c)
            off = 128 * t - CH * c if diag else 0
            strip0 = ps.tile([128, 2 * CH], F32, tag="scP0")
            strip1 = ps.tile([128, 2 * CH], F32, tag="scP1")
            strips = [strip0, strip1]
            # all 4 tops first, then all 4 bots: 4-way row-group concurrency
            for h in range(HPC):
                hsl = slice(32 * h, 32 * h + 32)
                pss = strips[h // 2][:, CH * (h % 2):CH * (h % 2 + 1)]
                nc.tensor.matmul(pss, kt_t[hsl, ksl], qt_t[hsl, qsl],
                                 start=True, stop=False,
                                 tile_position=(32 * h, 0))
            for h in range(HPC):
                hsl = slice(32 * h, 32 * h + 32)
                pss = strips[h // 2][:, CH * (h % 2):CH * (h % 2 + 1)]
                nc.tensor.matmul(pss, kt_b[hsl, ksl], qt_b[hsl, qsl],
                                 start=False, stop=True,
                                 tile_position=(32 * h, 0))
            for p in range(2):
                strip = strips[p]
                expt = epool.tile([128, 2 * CH], BF16, tag="expt")
                esrc = strip[:].rearrange("q (h x) -> q h x", x=CH)[:, :, off:]
                edst = expt[:].rearrange("q (h x) -> q h x", x=CH)[:, :, off:]
                nc.scalar.activation(edst, esrc, AF.Exp, scale=0.125)
                if diag:
                    # triangular mask on the 128-wide diagonal block
                    for hh in range(2):
                        blk = slice(CH * hh + off, CH * hh + off + 128)
                        nc.vector.tensor_mul(expt[:, blk], expt[:, blk], tri)
                for hh in range(2):
                    h = 2 * p + hh
                    nc.tensor.matmul(av[h][:, off:], v_sb[t][:, 65 * h:65 * h + 65],
                                     expt[:, CH * hh + off:CH * (hh + 1)],
                                     start=(t == 0), stop=(t == ntile - 1))
            if c + 1 < NCH and t in (1, 3):
                emit_qk_part(c + 1, t // 2)
        invs = []
        for h in range(HPC):
            inv = ipool.tile([1, CH], F32R, tag=f"inv{h}")
            with nc.allow_low_precision(reason="tf32 bcast of softmax denom"):
                nc.vector.reciprocal(inv, av[h][64:65, :])
            invs.append(inv)
        for p in range(2):
            bc2 = ps.tile([64, 2 * CH], F32, tag=f"scP{p}")
            for hh in range(2):
                h = 2 * p + hh
                nc.tensor.matmul(bc2[:, CH * hh:CH * (hh + 1)], ones_sb[:],
                                 invs[h][:], start=True, stop=True)
            bc_sb = ipool.tile([64, 2 * CH], F32, tag="bcsb")
            nc.vector.tensor_copy(bc_sb, bc2)
            for hh in range(2):
                h = 2 * p + hh
                nc.vector.tensor_mul(ho[h // 2][64 * (h % 2):64 * (h % 2) + 64, qsl],
                                     av[h][0:64, :], bc_sb[:, CH * hh:CH * (hh + 1)])
        if c + 1 < NCH:
            emit_v_tiles(c + 1)

        # ---- output projection for this chunk ----
        for lt in range(4 * c, 4 * c + 4):
            for oc in range(2):
                osl = slice(512 * oc, 512 * oc + 512)
                ps_o = ps.tile([128, 512], F32, tag=f"av{(2 * lt + oc) % 4}")
                for j in range(2):
                    nc.tensor.matmul(ps_o, ho[j][:, 128 * lt:128 * lt + 128],
                                     wo_sb[j][:, osl], start=(j == 0), stop=(j == 1))
                o_sb = opool.tile([128, 512], F32, tag="o")
                # alternate engines to balance DVE vs ACT load
                if (2 * lt + oc) % 2 == 0:
                    nc.vector.tensor_copy(o_sb, ps_o)
                else:
                    nc.scalar.copy(o_sb, ps_o)
                nc.sync.dma_start(out[128 * lt:128 * lt + 128, osl], o_sb)


def _build_nc():
    nc = bacc.Bacc("TRN2", target_bir_lowering=False, debug=False,
                   enable_asserts=False, num_devices=8)
    ins = {
        "xt": nc.dram_tensor("xt", [DM, L], F32, kind="ExternalInput").ap(),
        "wq": nc.dram_tensor("wq", [DM, 512], F32, kind="ExternalInput").ap(),
        "wv": nc.dram_tensor("wv", [DM, 256], F32, kind="ExternalInput").ap(),
        "wo": nc.dram_tensor("wo", [256, DM], F32, kind="ExternalInput").ap(),
        "cs": nc.dram_tensor("cs", [128, L], F32, kind="ExternalInput").ap(),
        "sn": nc.dram_tensor("sn", [128, L], F32, kind="ExternalInput").ap(),
    }
    outs = {"out": nc.dram_tensor("out", [L, DM], F32, kind="ExternalOutput").ap()}
    with tile.TileContext(nc) as tc:
        _attn_kernel(tc, outs, ins)
    nc.compile()
    return nc


def _host_shard(X, token_positions, Wqkv, Wout):
    """Build the 8 per-core input maps."""
    X = np.asarray(X, dtype=np.float32)
    Wqkv = np.asarray(Wqkv, dtype=np.float32)
    Wout = np.asarray(Wout, dtype=np.float32)
    pos = np.asarray(token_positions)

    # RoPE tables in float32 arithmetic to mirror the f32 reference
    k = np.arange(DK // 2, dtype=np.float32)
    inv_freq = (np.float32(1.0) /
                np.power(np.float32(THETA), (np.float32(2.0) * k) / np.float32(DK)))
    inv_freq = inv_freq.astype(np.float32)
    ang = (pos.astype(np.float32)[:, None, :] *
           inv_freq[None, :, None]).astype(np.float32)       # [B, 32, L]
    cos = np.cos(ang).astype(np.float32)
    sin = np.sin(ang).astype(np.float32)
    cs_all = np.tile(cos, (1, HPC, 1))  # [B, 128, L]
    sn_all = np.tile(sin, (1, HPC, 1))

    in_maps = []
    for core in range(8):
        b, g = divmod(core, HPC)
        heads = [HPC * g + hh for hh in range(HPC)]
        q_top, q_bot, k_top, k_bot = [], [], [], []
        for h in heads:
            base = DK * h
            q_top += [base + 2 * kk for kk in range(DK // 2)]
            q_bot += [base + 2 * kk + 1 for kk in range(DK // 2)]
            k_top += [DM + base + 2 * kk for kk in range(DK // 2)]
            k_bot += [DM + base + 2 * kk + 1 for kk in range(DK // 2)]
        wq_c = np.ascontiguousarray(Wqkv[q_top + q_bot + k_top + k_bot, :].T)
        v_rows = [2 * DM + DK * h + j for h in heads for j in range(DK)]
        wv_c = np.ascontiguousarray(Wqkv[v_rows, :].T)
        wo_c = np.ascontiguousarray(Wout[:, 256 * g:256 * (g + 1)].T)
        in_maps.append({
            "xt": np.ascontiguousarray(X[b].T),
            "wq": wq_c,
            "wv": wv_c,
            "wo": wo_c,
            "cs": np.ascontiguousarray(cs_all[b]),
            "sn": np.ascontiguousarray(sn_all[b]),
        })
    return in_maps


def kernel(X, token_positions, Wqkv, Wout, _trace=False):
    if "nc" not in _cache:
        _cache["nc"] = _build_nc()
    nc = _cache["nc"]
    in_maps = _host_shard(X, token_positions, Wqkv, Wout)
    res = run_bass_kernel_spmd(nc, in_maps, list(range(8)), trace=_trace)
    _cache["last_results"] = res
    out = np.zeros((B, L, DM), dtype=np.float32)
    for core in range(8):
        out[core // HPC] += res.results[core]["out"]
    return out



# revision 2
# speedup vs baseline: 1.1053x; 1.1053x over previous
"""Causal multi-head attention with RoPE on 8 Trainium2 NeuronCores.

Problem: B=2, L=2048, D_MODEL=1024, N_HEADS=16, D_K=64, theta=10000.
Sharding: data parallel on batch (2) x tensor parallel on heads (4 groups of
4 heads) = 8 cores. Each core computes its 4 heads' attention plus a partial
output projection; partials are summed on the host (Megatron row-parallel).

Per-core device design (v3):
- Q/K live head-contiguous in bf16: each head owns 64 partitions, with its
  RoPE top (even) / bot (odd) dims interleaved in 16-blocks so the rotation
  partner is always p^16 within a 32-quadrant.  RoPE is then 2 full-width
  DVE muls (cos / sign-folded sin) + one stream_shuffle(+-16) + one bf16
  add.  Scores become a SINGLE K=64 matmul per head-tile (the cost model
  charges a matmul by its moving size regardless of K, so the old K=32
  top/bot pair paid 2x), with causal column-slicing on diagonal tiles.
- PSUM: s0/s1 are 2-bank strips (2 heads of scoresT each), a0..a3 1-bank AV
  accumulators (with an appended ones-column on V giving the softmax denom).
  Projections use only s0/s1 (QA+QB / KA+KB / V0..V3 share single 1024-wide
  instances), so attention's AV tags never write-after-read-block on the
  projection pipeline.
- Emission order: proj(0), proj(1) up front; then per chunk c:
  attention(c) -> proj(c+2) (PE filler while ACT drains the exp backlog) ->
  normalize(c) -> output-projection(c).
"""
import numpy as np
from contextlib import ExitStack

import concourse.bacc as bacc
import concourse.bass as bass
import concourse.mybir as mybir
import concourse.tile as tile
from concourse._compat import with_exitstack
from concourse.bass_utils import run_bass_kernel_spmd

F32 = mybir.dt.float32
F32R = mybir.dt.float32r
BF16 = mybir.dt.bfloat16

B, L, DM, NH, DK = 2, 2048, 1024, 16, 64
HPC = 4              # heads per core
THETA = 10000.0
CH = 512             # q/l chunk
NT = L // 128        # 16 kv tiles
NCH = L // CH        # 4 chunks

_cache = {}

# stream_shuffle mask: swap 16-blocks within each 32-quadrant (p <-> p^16)
_SWAP16 = list(range(16, 32)) + list(range(16))


@with_exitstack
def _attn_kernel(ctx: ExitStack, tc: tile.TileContext, outs, ins):
    nc = tc.nc
    xt, wq, wv, wo = ins["xt"], ins["wq"], ins["wv"], ins["wo"]
    cs, sn = ins["cs"], ins["sn"]
    out = outs["out"]
    AF = mybir.ActivationFunctionType

    consts = ctx.enter_context(tc.tile_pool(name="consts", bufs=1))
    persist = ctx.enter_context(tc.tile_pool(name="persist", bufs=1))
    ps = ctx.enter_context(tc.tile_pool(name="ps", bufs=1, space="PSUM"))
    xtp = ctx.enter_context(tc.tile_pool(name="xtp", bufs=18))
    csp = ctx.enter_context(tc.tile_pool(name="csp", bufs=3))
    ropet = ctx.enter_context(tc.tile_pool(name="ropet", bufs=2))
    epool = ctx.enter_context(tc.tile_pool(name="epool", bufs=4))
    ipool = ctx.enter_context(tc.tile_pool(name="ipool", bufs=2))
    opool = ctx.enter_context(tc.tile_pool(name="opool", bufs=4))

    # ---- weights / constants ----
    # wq and the first xt chunk are interleaved per-d so the d=0..7
    # accumulation chain starts as soon as each pair lands
    wq_sb = []
    xt0 = []
    for d in range(8):
        t_wq = consts.tile([128, 512], F32R, tag=f"wq{d}")
        nc.sync.dma_start(t_wq, wq[128 * d:128 * d + 128, :].bitcast(F32R))
        wq_sb.append(t_wq)
        t_x0 = xtp.tile([128, CH], F32R, tag="xt")
        nc.sync.dma_start(t_x0, xt[128 * d:128 * d + 128, 0:CH].bitcast(F32R))
        xt0.append(t_x0)
    ones_f = consts.tile([1, 64], F32)
    nc.vector.memset(ones_f, 1.0)
    ones_sb = consts.tile([1, 64], F32R)
    nc.vector.tensor_copy(ones_sb, ones_f)
    # lower-triangular keep-mask (keep iff q_local >= kv_local)
    tri = consts.tile([128, 128], BF16)
    nc.vector.memset(tri, 1.0)
    nc.gpsimd.affine_select(tri, tri, pattern=[[1, 128]],
                            compare_op=mybir.AluOpType.is_ge, fill=0.0,
                            base=0, channel_multiplier=-1)

    # persistent activations: head-contiguous RoPE'd Q/K in bf16
    q2a = persist.tile([128, L], BF16)   # heads 0,1
    q2b = persist.tile([128, L], BF16)   # heads 2,3
    k2a = persist.tile([128, L], BF16)
    k2b = persist.tile([128, L], BF16)
    v_sb = []
    for t in range(NT):
        t_v = persist.tile([128, HPC * 65], BF16, tag=f"v{t}")
        v_sb.append(t_v)
        nc.vector.memset(t_v[:, 64:HPC * 65:65], 1.0)
    ho = []
    for j in range(2):
        t_ho = persist.tile([128, L], F32R, tag=f"ho{j}")
        ho.append(t_ho)

    wv_sb = []
    wo_sb = []

    def _load_wv():
        res = []
        for d in range(8):
            t_wv = consts.tile([128, 256], F32R, tag=f"wv{d}")
            nc.sync.dma_start(t_wv, wv[128 * d:128 * d + 128, :].bitcast(F32R))
            res.append(t_wv)
        return res

    def _load_wo():
        res = []
        for j in range(2):
            t_wo = consts.tile([128, DM], F32R, tag=f"wo{j}")
            nc.sync.dma_start(t_wo, wo[128 * j:128 * j + 128, :].bitcast(F32R))
            res.append(t_wo)
        return res

    state = {}

    def emit_xt_dmas(c):
        lsl = slice(CH * c, CH * (c + 1))
        xt_c = []
        for d in range(8):
            t_x = xtp.tile([128, CH], F32R, tag="xt")
            nc.sync.dma_start(t_x, xt[128 * d:128 * d + 128, lsl].bitcast(F32R))
            xt_c.append(t_x)
        cs_c = csp.tile([128, CH], F32, tag="cs")
        nc.sync.dma_start(cs_c, cs[:, lsl])
        sn_c = csp.tile([128, CH], F32, tag="sn")
        nc.sync.dma_start(sn_c, sn[:, lsl])
        state[c] = (xt_c, cs_c, sn_c)

    def emit_rope(psrc, cs_c, sn_c, dst, lsl):
        """RoPE a [128, CH] PSUM pair-tile into bf16 dst[:, lsl]."""
        tmpc = ropet.tile([128, CH], BF16, tag="tc")
        tmps = ropet.tile([128, CH], BF16, tag="ts")
        tmpw = ropet.tile([128, CH], BF16, tag="tw")
        nc.vector.tensor_mul(tmpc, psrc, cs_c)
        nc.vector.tensor_mul(tmps, psrc, sn_c)
        nc.vector.stream_shuffle(tmpw, tmps, mask=_SWAP16)
        nc.vector.tensor_add(dst[:, lsl], tmpc, tmpw)

    def emit_proj(c):
        """QKV projection + RoPE for chunk c (uses only s0/s1 PSUM tags)."""
        xt_c, cs_c, sn_c = state[c]
        lsl = slice(CH * c, CH * (c + 1))
        # Q: pairs A (cols 0:128 of wq) and B (128:256) share one instance
        psq = ps.tile([128, 2 * CH], F32, tag="scP0")
        for d in range(8):
            nc.tensor.matmul(psq[:, 0:CH], wq_sb[d][:, 0:128], xt_c[d][:],
                             start=(d == 0), stop=(d == 7))
            nc.tensor.matmul(psq[:, CH:2 * CH], wq_sb[d][:, 128:256],
                             xt_c[d][:], start=(d == 0), stop=(d == 7))
        emit_rope(psq[:, 0:CH], cs_c, sn_c, q2a, lsl)
        emit_rope(psq[:, CH:2 * CH], cs_c, sn_c, q2b, lsl)
        # K
        psk = ps.tile([128, 2 * CH], F32, tag="scP1")
        for d in range(8):
            nc.tensor.matmul(psk[:, 0:CH], wq_sb[d][:, 256:384], xt_c[d][:],
                             start=(d == 0), stop=(d == 7))
            nc.tensor.matmul(psk[:, CH:2 * CH], wq_sb[d][:, 384:512],
                             xt_c[d][:], start=(d == 0), stop=(d == 7))
        emit_rope(psk[:, 0:CH], cs_c, sn_c, k2a, lsl)
        emit_rope(psk[:, CH:2 * CH], cs_c, sn_c, k2b, lsl)
        # V: 4 l-tiles of [128, 256] share one instance
        psv = ps.tile([128, 2 * CH], F32, tag="scP0")
        for i in range(4):
            t = 4 * c + i
            lo = 128 * i
            for d in range(8):
                nc.tensor.matmul(psv[:, 256 * i:256 * i + 256],
                                 xt_c[d][:, lo:lo + 128], wv_sb[d][:],
                                 start=(d == 0), stop=(d == 7))
        for i in range(4):
            t = 4 * c + i
            vdst = v_sb[t][:].rearrange("p (h x) -> p h x", x=65)[:, :, 0:64]
            vsrc = psv[:, 256 * i:256 * i + 256].rearrange(
                "p (h x) -> p h x", x=64)
            nc.vector.tensor_copy(vdst, vsrc)

    def emit_attention(c):
        """Scores + exp + AV accumulate for chunk c."""
        qsl = slice(CH * c, CH * (c + 1))
        av = []
        for h in range(HPC):
            t_av = ps.tile([65, CH], F32, tag=f"av{h}")
            av.append(t_av)
        ntile = 4 * c + 4
        for t in range(ntile):
            ksl = slice(128 * t, 128 * t + 128)
            diag = (t // 4 == c)
            off = 128 * t - CH * c if diag else 0
            strip0 = ps.tile([128, 2 * CH], F32, tag="scP0")
            strip1 = ps.tile([128, 2 * CH], F32, tag="scP1")
            strips = [strip0, strip1]
            for h in range(HPC):
                k2 = (k2a, k2b)[h // 2]
                q2 = (q2a, q2b)[h // 2]
                hsl = slice(64 * (h % 2), 64 * (h % 2) + 64)
                pss = strips[h // 2][:, CH * (h % 2) + off:CH * (h % 2 + 1)]
                nc.tensor.matmul(pss, k2[hsl, ksl],
                                 q2[hsl, CH * c + off:CH * (c + 1)],
                                 start=True, stop=True,
                                 tile_position=(64 * (h % 2), 0))
            for p in range(2):
                strip = strips[p]
                expt = epool.tile([128, 2 * CH], BF16, tag="expt")
                esrc = strip[:].rearrange("q (h x) -> q h x", x=CH)[:, :, off:]
                edst = expt[:].rearrange("q (h x) -> q h x", x=CH)[:, :, off:]
                nc.scalar.activation(edst, esrc, AF.Exp, scale=0.125)
                if diag:
                    # triangular mask on the 128-wide diagonal block
                    for hh in range(2):
                        blk = slice(CH * hh + off, CH * hh + off + 128)
                        nc.vector.tensor_mul(expt[:, blk], expt[:, blk], tri)
                for hh in range(2):
                    h = 2 * p + hh
                    nc.tensor.matmul(av[h][:, off:],
                                     v_sb[t][:, 65 * h:65 * h + 65],
                                     expt[:, CH * hh + off:CH * (hh + 1)],
                                     start=(t == 0), stop=(t == ntile - 1))
        return av

    def emit_norm(c, av):
        """Softmax normalization: ho = av * (1/denom) for chunk c."""
        qsl = slice(CH * c, CH * (c + 1))
        invs = []
        for h in range(HPC):
            inv = ipool.tile([1, CH], F32R, tag=f"inv{h}")
            with nc.allow_low_precision(reason="tf32 bcast of softmax denom"):
                nc.vector.reciprocal(inv, av[h][64:65, :])
            invs.append(inv)
        for p in range(2):
            bc2 = ps.tile([64, 2 * CH], F32, tag=f"scP{p}")
            for hh in range(2):
                h = 2 * p + hh
                nc.tensor.matmul(bc2[:, CH * hh:CH * (hh + 1)], ones_sb[:],
                                 invs[h][:], start=True, stop=True)
            bc_sb = ipool.tile([64, 2 * CH], F32, tag="bcsb")
            nc.vector.tensor_copy(bc_sb, bc2)
            for hh in range(2):
                h = 2 * p + hh
                nc.vector.tensor_mul(
                    ho[h // 2][64 * (h % 2):64 * (h % 2) + 64, qsl],
                    av[h][0:64, :], bc_sb[:, CH * hh:CH * (hh + 1)])

    def emit_outproj(c):
        for lt in range(4 * c, 4 * c + 4):
            for oc in range(2):
                osl = slice(512 * oc, 512 * oc + 512)
                ps_o = ps.tile([128, 512], F32, tag=f"av{(2 * lt + oc) % 4}")
                for j in range(2):
                    nc.tensor.matmul(ps_o, ho[j][:, 128 * lt:128 * lt + 128],
                                     wo_sb[j][:, osl],
                                     start=(j == 0), stop=(j == 1))
                o_sb = opool.tile([128, 512], F32, tag="o")
                nc.vector.tensor_copy(o_sb, ps_o)
                nc.sync.dma_start(out[128 * lt:128 * lt + 128, osl], o_sb)

    # ---- software pipeline ----
    cs_0 = csp.tile([128, CH], F32, tag="cs")
    nc.sync.dma_start(cs_0, cs[:, 0:CH])
    sn_0 = csp.tile([128, CH], F32, tag="sn")
    nc.sync.dma_start(sn_0, sn[:, 0:CH])
    state[0] = (xt0, cs_0, sn_0)
    wv_sb.extend(_load_wv())
    wo_sb.extend(_load_wo())
    emit_xt_dmas(1)
    emit_proj(0)
    emit_xt_dmas(2)
    emit_proj(1)
    for c in range(NCH):
        av = emit_attention(c)
        if c + 2 < NCH:
            emit_xt_dmas(c + 3) if c + 3 < NCH else None
            emit_proj(c + 2)
        emit_norm(c, av)
        emit_outproj(c)


def _build_nc():
    nc = bacc.Bacc("TRN2", target_bir_lowering=False, debug=False,
                   enable_asserts=False, num_devices=8)
    ins = {
        "xt": nc.dram_tensor("xt", [DM, L], F32, kind="ExternalInput").ap(),
        "wq": nc.dram_tensor("wq", [DM, 512], F32, kind="ExternalInput").ap(),
        "wv": nc.dram_tensor("wv", [DM, 256], F32, kind="ExternalInput").ap(),
        "wo": nc.dram_tensor("wo", [256, DM], F32, kind="ExternalInput").ap(),
        "cs": nc.dram_tensor("cs", [128, L], F32, kind="ExternalInput").ap(),
        "sn": nc.dram_tensor("sn", [128, L], F32, kind="ExternalInput").ap(),
    }
    outs = {"out": nc.dram_tensor("out", [L, DM], F32, kind="ExternalOutput").ap()}
    with tile.TileContext(nc) as tc:
        _attn_kernel(tc, outs, ins)
    nc.compile()
    return nc


def _host_shard(X, token_positions, Wqkv, Wout):
    """Build the 8 per-core input maps."""
    X = np.asarray(X, dtype=np.float32)
    Wqkv = np.asarray(Wqkv, dtype=np.float32)
    Wout = np.asarray(Wout, dtype=np.float32)
    pos = np.asarray(token_positions)

    # Per-head dim order: [e0..e15, o0..o15, e16..e31, o16..o31] so the RoPE
    # partner (even<->odd of the same freq) is p^16 within a 32-quadrant.
    def head_order(base):
        return ([base + 2 * k for k in range(16)] +
                [base + 2 * k + 1 for k in range(16)] +
                [base + 2 * k for k in range(16, 32)] +
                [base + 2 * k + 1 for k in range(16, 32)])

    # RoPE tables in float32 arithmetic to mirror the f32 reference
    k = np.arange(DK // 2, dtype=np.float32)
    inv_freq = (np.float32(1.0) /
                np.power(np.float32(THETA), (np.float32(2.0) * k) / np.float32(DK)))
    inv_freq = inv_freq.astype(np.float32)
    # freq index per partition within a 64-block: [0:16]=f0..15, [16:32]=f0..15,
    # [32:48]=f16..31, [48:64]=f16..31; sign +1 on top blocks, -1 on bot blocks
    fidx = np.concatenate([np.arange(16), np.arange(16),
                           np.arange(16, 32), np.arange(16, 32)])
    sgn = np.concatenate([np.ones(16), -np.ones(16),
                          np.ones(16), -np.ones(16)]).astype(np.float32)
    fidx = np.tile(fidx, 2)           # 128 partitions (2 heads per tile)
    sgn = np.tile(sgn, 2)
    ang = (pos.astype(np.float32)[:, None, :] *
           inv_freq[fidx][None, :, None]).astype(np.float32)   # [B, 128, L]
    cs_all = np.cos(ang).astype(np.float32)
    sn_all = (np.sin(ang) * sgn[None, :, None]).astype(np.float32)

    in_maps = []
    for core in range(8):
        b, g = divmod(core, HPC)
        heads = [HPC * g + hh for hh in range(HPC)]
        q_rows, k_rows = [], []
        for h in heads:
            q_rows += head_order(DK * h)
            k_rows += head_order(DM + DK * h)
        wq_c = np.ascontiguousarray(Wqkv[q_rows + k_rows, :].T)
        v_rows = [2 * DM + DK * h + j for h in heads for j in range(DK)]
        wv_c = np.ascontiguousarray(Wqkv[v_rows, :].T)
        wo_c = np.ascontiguousarray(Wout[:, 256 * g:256 * (g + 1)].T)
        in_maps.append({
            "xt": np.ascontiguousarray(X[b].T),
            "wq": wq_c,
            "wv": wv_c,
            "wo": wo_c,
            "cs": np.ascontiguousarray(cs_all[b]),
            "sn": np.ascontiguousarray(sn_all[b]),
        })
    return in_maps


def kernel(X, token_positions, Wqkv, Wout, _trace=False):
    if "nc" not in _cache:
        _cache["nc"] = _build_nc()
    nc = _cache["nc"]
    in_maps = _host_shard(X, token_positions, Wqkv, Wout)
    res = run_bass_kernel_spmd(nc, in_maps, list(range(8)), trace=_trace)
    _cache["last_results"] = res
    out = np.zeros((B, L, DM), dtype=np.float32)
    for core in range(8):
        out[core // HPC] += res.results[core]["out"]
    return out


# revision 14
# speedup vs baseline: 1.1246x; 1.0174x over previous
"""Causal multi-head attention with RoPE on 8 Trainium2 NeuronCores.

Problem: B=2, L=2048, D_MODEL=1024, N_HEADS=16, D_K=64, theta=10000.
Sharding: data parallel on batch (2) x tensor parallel on heads (4 groups of
4 heads) = 8 cores. Each core computes its 4 heads' attention plus a partial
output projection; partials are summed on the host (Megatron row-parallel).

Per-core device design (v3):
- Q/K live head-contiguous in bf16: each head owns 64 partitions, with its
  RoPE top (even) / bot (odd) dims interleaved in 16-blocks so the rotation
  partner is always p^16 within a 32-quadrant.  RoPE is then 2 full-width
  DVE muls (cos / sign-folded sin) + one stream_shuffle(+-16) + one bf16
  add.  Scores become a SINGLE K=64 matmul per head-tile (the cost model
  charges a matmul by its moving size regardless of K, so the old K=32
  top/bot pair paid 2x), with causal column-slicing on diagonal tiles.
- PSUM: s0/s1 are 2-bank strips (2 heads of scoresT each), a0..a3 1-bank AV
  accumulators (with an appended ones-column on V giving the softmax denom).
  Projections use only s0/s1 (QA+QB / KA+KB / V0..V3 share single 1024-wide
  instances), so attention's AV tags never write-after-read-block on the
  projection pipeline.
- Emission order: proj(0), proj(1) up front; then per chunk c:
  attention(c) -> proj(c+2) (PE filler while ACT drains the exp backlog) ->
  normalize(c) -> output-projection(c).
"""
import numpy as np
import ml_dtypes
from contextlib import ExitStack

import concourse.bacc as bacc
import concourse.bass as bass
import concourse.mybir as mybir
import concourse.tile as tile
from concourse._compat import with_exitstack
from concourse.bass_utils import run_bass_kernel_spmd

F32 = mybir.dt.float32
F32R = mybir.dt.float32r
BF16 = mybir.dt.bfloat16

B, L, DM, NH, DK = 2, 2048, 1024, 16, 64
HPC = 4              # heads per core
THETA = 10000.0
CH = 512             # q/l chunk
NT = L // 128        # 16 kv tiles
NCH = L // CH        # 4 chunks

_cache = {}

# stream_shuffle mask: swap 16-blocks within each 32-quadrant (p <-> p^16)
_SWAP16 = list(range(16, 32)) + list(range(16))


@with_exitstack
def _attn_kernel(ctx: ExitStack, tc: tile.TileContext, outs, ins):
    nc = tc.nc
    xt, wq, wv, wo = ins["xt"], ins["wq"], ins["wv"], ins["wo"]
    cs, sn = ins["cs"], ins["sn"]
    out = outs["out"]
    AF = mybir.ActivationFunctionType

    consts = ctx.enter_context(tc.tile_pool(name="consts", bufs=1))
    persist = ctx.enter_context(tc.tile_pool(name="persist", bufs=1))
    ps = ctx.enter_context(tc.tile_pool(name="ps", bufs=1, space="PSUM"))
    xtp = ctx.enter_context(tc.tile_pool(name="xtp", bufs=18))
    csp = ctx.enter_context(tc.tile_pool(name="csp", bufs=3))
    ropet = ctx.enter_context(tc.tile_pool(name="ropet", bufs=2))
    epool = ctx.enter_context(tc.tile_pool(name="epool", bufs=4))
    ipool = ctx.enter_context(tc.tile_pool(name="ipool", bufs=2))
    opool = ctx.enter_context(tc.tile_pool(name="opool", bufs=4))

    # ---- weights / constants ----
    # wq and the first xt chunk are interleaved per-d so the d=0..7
    # accumulation chain starts as soon as each pair lands
    wq_sb = []
    xt0 = []
    for d in range(8):
        t_wq = consts.tile([128, 512], BF16, tag=f"wq{d}")
        nc.sync.dma_start(t_wq, wq[128 * d:128 * d + 128, :])
        wq_sb.append(t_wq)
        t_x0 = xtp.tile([128, CH], BF16, tag="xt")
        nc.sync.dma_start(t_x0, xt[128 * d:128 * d + 128, 0:CH])
        xt0.append(t_x0)
    ones_f = consts.tile([1, 64], F32)
    nc.vector.memset(ones_f, 1.0)
    ones_sb = consts.tile([1, 64], F32R)
    nc.vector.tensor_copy(ones_sb, ones_f)
    # lower-triangular keep-mask (keep iff q_local >= kv_local)
    tri = consts.tile([128, 128], BF16)
    nc.vector.memset(tri, 1.0)
    nc.gpsimd.affine_select(tri, tri, pattern=[[1, 128]],
                            compare_op=mybir.AluOpType.is_ge, fill=0.0,
                            base=0, channel_multiplier=-1)

    # persistent activations: head-contiguous RoPE'd Q/K in bf16
    q2a = persist.tile([128, L], BF16)   # heads 0,1
    q2b = persist.tile([128, L], BF16)   # heads 2,3
    k2a = persist.tile([128, L], BF16)
    k2b = persist.tile([128, L], BF16)
    v_sb = []
    for t in range(NT):
        t_v = persist.tile([128, HPC * 65], BF16, tag=f"v{t}")
        v_sb.append(t_v)
        nc.vector.memset(t_v[:, 64:HPC * 65:65], 1.0)
    ho = []
    for j in range(2):
        t_ho = persist.tile([128, L], F32R, tag=f"ho{j}")
        ho.append(t_ho)

    wv_sb = []
    wo_sb = []

    def _load_wv():
        res = []
        for d in range(8):
            t_wv = consts.tile([128, 256], BF16, tag=f"wv{d}")
            nc.sync.dma_start(t_wv, wv[128 * d:128 * d + 128, :])
            res.append(t_wv)
        return res

    def _load_wo():
        res = []
        for j in range(2):
            t_wo = consts.tile([128, DM], F32R, tag=f"wo{j}")
            nc.sync.dma_start(t_wo, wo[128 * j:128 * j + 128, :].bitcast(F32R))
            res.append(t_wo)
        return res

    state = {}

    def emit_xt_dmas(c):
        lsl = slice(CH * c, CH * (c + 1))
        xt_c = []
        for d in range(8):
            t_x = xtp.tile([128, CH], BF16, tag="xt")
            nc.sync.dma_start(t_x, xt[128 * d:128 * d + 128, lsl])
            xt_c.append(t_x)
        cs_c = csp.tile([128, CH], F32, tag="cs")
        nc.sync.dma_start(cs_c, cs[:, lsl])
        sn_c = csp.tile([128, CH], F32, tag="sn")
        nc.sync.dma_start(sn_c, sn[:, lsl])
        state[c] = (xt_c, cs_c, sn_c)

    def emit_rope(psrc, cs_c, sn_c, dst, lsl):
        """RoPE a [128, CH] PSUM pair-tile into bf16 dst[:, lsl]."""
        tmpc = ropet.tile([128, CH], BF16, tag="tc")
        tmps = ropet.tile([128, CH], BF16, tag="ts")
        tmpw = ropet.tile([128, CH], BF16, tag="tw")
        nc.vector.tensor_mul(tmpc, psrc, cs_c)
        nc.vector.tensor_mul(tmps, psrc, sn_c)
        nc.vector.stream_shuffle(tmpw, tmps, mask=_SWAP16)
        nc.vector.tensor_add(dst[:, lsl], tmpc, tmpw)

    def emit_proj(c):
        """QKV projection + RoPE for chunk c (uses only s0/s1 PSUM tags)."""
        xt_c, cs_c, sn_c = state[c]
        lsl = slice(CH * c, CH * (c + 1))
        # Q: pairs A (cols 0:128 of wq) and B (128:256) share one instance
        psq = ps.tile([128, 2 * CH], F32, tag="scP0")
        for d in range(8):
            nc.tensor.matmul(psq[:, 0:CH], wq_sb[d][:, 0:128], xt_c[d][:],
                             start=(d == 0), stop=(d == 7))
            nc.tensor.matmul(psq[:, CH:2 * CH], wq_sb[d][:, 128:256],
                             xt_c[d][:], start=(d == 0), stop=(d == 7))
        emit_rope(psq[:, 0:CH], cs_c, sn_c, q2a, lsl)
        emit_rope(psq[:, CH:2 * CH], cs_c, sn_c, q2b, lsl)
        # K
        psk = ps.tile([128, 2 * CH], F32, tag="scP1")
        for d in range(8):
            nc.tensor.matmul(psk[:, 0:CH], wq_sb[d][:, 256:384], xt_c[d][:],
                             start=(d == 0), stop=(d == 7))
            nc.tensor.matmul(psk[:, CH:2 * CH], wq_sb[d][:, 384:512],
                             xt_c[d][:], start=(d == 0), stop=(d == 7))
        emit_rope(psk[:, 0:CH], cs_c, sn_c, k2a, lsl)
        emit_rope(psk[:, CH:2 * CH], cs_c, sn_c, k2b, lsl)
        # V: 4 l-tiles of [128, 256] share one instance
        psv = ps.tile([128, 2 * CH], F32, tag="scP0")
        for i in range(4):
            t = 4 * c + i
            lo = 128 * i
            for d in range(8):
                nc.tensor.matmul(psv[:, 256 * i:256 * i + 256],
                                 xt_c[d][:, lo:lo + 128], wv_sb[d][:],
                                 start=(d == 0), stop=(d == 7))
        for i in range(4):
            t = 4 * c + i
            vdst = v_sb[t][:].rearrange("p (h x) -> p h x", x=65)[:, :, 0:64]
            vsrc = psv[:, 256 * i:256 * i + 256].rearrange(
                "p (h x) -> p h x", x=64)
            nc.vector.tensor_copy(vdst, vsrc)

    def emit_attention(c):
        """Scores + exp + AV accumulate for chunk c."""
        qsl = slice(CH * c, CH * (c + 1))
        av = []
        for h in range(HPC):
            t_av = ps.tile([65, CH], F32, tag=f"av{h}")
            av.append(t_av)
        ntile = 4 * c + 4
        for t in range(ntile):
            ksl = slice(128 * t, 128 * t + 128)
            diag = (t // 4 == c)
            off = 128 * t - CH * c if diag else 0
            strip0 = ps.tile([128, 2 * CH], F32, tag="scP0")
            strip1 = ps.tile([128, 2 * CH], F32, tag="scP1")
            strips = [strip0, strip1]
            for h in range(HPC):
                k2 = (k2a, k2b)[h // 2]
                q2 = (q2a, q2b)[h // 2]
                hsl = slice(64 * (h % 2), 64 * (h % 2) + 64)
                pss = strips[h // 2][:, CH * (h % 2) + off:CH * (h % 2 + 1)]
                nc.tensor.matmul(pss, k2[hsl, ksl],
                                 q2[hsl, CH * c + off:CH * (c + 1)],
                                 start=True, stop=True,
                                 tile_position=(64 * (h % 2), 0))
            for p in range(2):
                strip = strips[p]
                expt = epool.tile([128, 2 * CH], BF16, tag="expt")
                esrc = strip[:].rearrange("q (h x) -> q h x", x=CH)[:, :, off:]
                edst = expt[:].rearrange("q (h x) -> q h x", x=CH)[:, :, off:]
                nc.scalar.activation(edst, esrc, AF.Exp, scale=0.125)
                if diag:
                    # triangular mask on the 128-wide diagonal block
                    for hh in range(2):
                        blk = slice(CH * hh + off, CH * hh + off + 128)
                        nc.vector.tensor_mul(expt[:, blk], expt[:, blk], tri)
                for hh in range(2):
                    h = 2 * p + hh
                    nc.tensor.matmul(av[h][:, off:],
                                     v_sb[t][:, 65 * h:65 * h + 65],
                                     expt[:, CH * hh + off:CH * (hh + 1)],
                                     start=(t == 0), stop=(t == ntile - 1))
        return av

    def emit_norm(c, av):
        """Softmax normalization: ho = av * (1/denom) for chunk c."""
        qsl = slice(CH * c, CH * (c + 1))
        invs = []
        for h in range(HPC):
            inv = ipool.tile([1, CH], F32R, tag=f"inv{h}")
            with nc.allow_low_precision(reason="tf32 bcast of softmax denom"):
                nc.vector.reciprocal(inv, av[h][64:65, :])
            invs.append(inv)
        for p in range(2):
            bc2 = ps.tile([64, 2 * CH], F32, tag=f"scP{p}")
            for hh in range(2):
                h = 2 * p + hh
                nc.tensor.matmul(bc2[:, CH * hh:CH * (hh + 1)], ones_sb[:],
                                 invs[h][:], start=True, stop=True)
            bc_sb = ipool.tile([64, 2 * CH], F32, tag="bcsb")
            nc.vector.tensor_copy(bc_sb, bc2)
            for hh in range(2):
                h = 2 * p + hh
                nc.vector.tensor_mul(
                    ho[h // 2][64 * (h % 2):64 * (h % 2) + 64, qsl],
                    av[h][0:64, :], bc_sb[:, CH * hh:CH * (hh + 1)])

    def emit_outproj(c):
        for lt in range(4 * c, 4 * c + 4):
            for oc in range(2):
                osl = slice(512 * oc, 512 * oc + 512)
                ps_o = ps.tile([128, 512], F32, tag=f"av{(2 * lt + oc) % 4}")
                for j in range(2):
                    nc.tensor.matmul(ps_o, ho[j][:, 128 * lt:128 * lt + 128],
                                     wo_sb[j][:, osl],
                                     start=(j == 0), stop=(j == 1))
                o_sb = opool.tile([128, 512], F32, tag="o")
                nc.vector.tensor_copy(o_sb, ps_o)
                nc.sync.dma_start(out[128 * lt:128 * lt + 128, osl], o_sb)

    # ---- software pipeline ----
    cs_0 = csp.tile([128, CH], F32, tag="cs")
    nc.sync.dma_start(cs_0, cs[:, 0:CH])
    sn_0 = csp.tile([128, CH], F32, tag="sn")
    nc.sync.dma_start(sn_0, sn[:, 0:CH])
    state[0] = (xt0, cs_0, sn_0)
    wv_sb.extend(_load_wv())
    emit_xt_dmas(1)
    emit_proj(0)
    wo_sb.extend(_load_wo())
    emit_xt_dmas(2)
    emit_proj(1)
    for c in range(NCH):
        av = emit_attention(c)
        if c + 2 < NCH:
            emit_xt_dmas(c + 3) if c + 3 < NCH else None
            emit_proj(c + 2)
        emit_norm(c, av)
        emit_outproj(c)


def _build_nc():
    nc = bacc.Bacc("TRN2", target_bir_lowering=False, debug=False,
                   enable_asserts=False, num_devices=8)
    ins = {
        "xt": nc.dram_tensor("xt", [DM, L], BF16, kind="ExternalInput").ap(),
        "wq": nc.dram_tensor("wq", [DM, 512], BF16, kind="ExternalInput").ap(),
        "wv": nc.dram_tensor("wv", [DM, 256], BF16, kind="ExternalInput").ap(),
        "wo": nc.dram_tensor("wo", [256, DM], F32, kind="ExternalInput").ap(),
        "cs": nc.dram_tensor("cs", [128, L], F32, kind="ExternalInput").ap(),
        "sn": nc.dram_tensor("sn", [128, L], F32, kind="ExternalInput").ap(),
    }
    outs = {"out": nc.dram_tensor("out", [L, DM], F32, kind="ExternalOutput").ap()}
    with tile.TileContext(nc) as tc:
        _attn_kernel(tc, outs, ins)
    nc.compile()
    return nc


def _host_shard(X, token_positions, Wqkv, Wout):
    """Build the 8 per-core input maps."""
    X = np.asarray(X, dtype=np.float32)
    Wqkv = np.asarray(Wqkv, dtype=np.float32)
    Wout = np.asarray(Wout, dtype=np.float32)
    pos = np.asarray(token_positions)

    # Per-head dim order: [e0..e15, o0..o15, e16..e31, o16..o31] so the RoPE
    # partner (even<->odd of the same freq) is p^16 within a 32-quadrant.
    def head_order(base):
        return ([base + 2 * k for k in range(16)] +
                [base + 2 * k + 1 for k in range(16)] +
                [base + 2 * k for k in range(16, 32)] +
                [base + 2 * k + 1 for k in range(16, 32)])

    # RoPE tables in float32 arithmetic to mirror the f32 reference
    k = np.arange(DK // 2, dtype=np.float32)
    inv_freq = (np.float32(1.0) /
                np.power(np.float32(THETA), (np.float32(2.0) * k) / np.float32(DK)))
    inv_freq = inv_freq.astype(np.float32)
    # freq index per partition within a 64-block: [0:16]=f0..15, [16:32]=f0..15,
    # [32:48]=f16..31, [48:64]=f16..31; sign +1 on top blocks, -1 on bot blocks
    fidx = np.concatenate([np.arange(16), np.arange(16),
                           np.arange(16, 32), np.arange(16, 32)])
    sgn = np.concatenate([np.ones(16), -np.ones(16),
                          np.ones(16), -np.ones(16)]).astype(np.float32)
    fidx = np.tile(fidx, 2)           # 128 partitions (2 heads per tile)
    sgn = np.tile(sgn, 2)
    ang = (pos.astype(np.float32)[:, None, :] *
           inv_freq[fidx][None, :, None]).astype(np.float32)   # [B, 128, L]
    cs_all = np.cos(ang).astype(np.float32)
    sn_all = (np.sin(ang) * sgn[None, :, None]).astype(np.float32)

    in_maps = []
    for core in range(8):
        b, g = divmod(core, HPC)
        heads = [HPC * g + hh for hh in range(HPC)]
        q_rows, k_rows = [], []
        for h in heads:
            q_rows += head_order(DK * h)
            k_rows += head_order(DM + DK * h)
        wq_c = np.ascontiguousarray(Wqkv[q_rows + k_rows, :].T
                                    .astype(ml_dtypes.bfloat16))
        v_rows = [2 * DM + DK * h + j for h in heads for j in range(DK)]
        wv_c = np.ascontiguousarray(Wqkv[v_rows, :].T.astype(ml_dtypes.bfloat16))
        wo_c = np.ascontiguousarray(Wout[:, 256 * g:256 * (g + 1)].T)
        in_maps.append({
            "xt": np.ascontiguousarray(X[b].T.astype(ml_dtypes.bfloat16)),
            "wq": wq_c,
            "wv": wv_c,
            "wo": wo_c,
            "cs": np.ascontiguousarray(cs_all[b]),
            "sn": np.ascontiguousarray(sn_all[b]),
        })
    return in_maps


def kernel(X, token_positions, Wqkv, Wout, _trace=False):
    if "nc" not in _cache:
        _cache["nc"] = _build_nc()
    nc = _cache["nc"]
    in_maps = _host_shard(X, token_positions, Wqkv, Wout)
    res = run_bass_kernel_spmd(nc, in_maps, list(range(8)), trace=_trace)
    _cache["last_results"] = res
    out = np.zeros((B, L, DM), dtype=np.float32)
    for core in range(8):
        out[core // HPC] += res.results[core]["out"]
    return out


# revision 19
# speedup vs baseline: 1.1362x; 1.0103x over previous
"""Causal multi-head attention with RoPE on 8 Trainium2 NeuronCores.

Problem: B=2, L=2048, D_MODEL=1024, N_HEADS=16, D_K=64, theta=10000.
Sharding: data parallel on batch (2) x tensor parallel on heads (4 groups of
4 heads) = 8 cores. Each core computes its 4 heads' attention plus a partial
output projection; partials are summed on the host (Megatron row-parallel).

Per-core device design (v3):
- Q/K live head-contiguous in bf16: each head owns 64 partitions, with its
  RoPE top (even) / bot (odd) dims interleaved in 16-blocks so the rotation
  partner is always p^16 within a 32-quadrant.  RoPE is then 2 full-width
  DVE muls (cos / sign-folded sin) + one stream_shuffle(+-16) + one bf16
  add.  Scores become a SINGLE K=64 matmul per head-tile (the cost model
  charges a matmul by its moving size regardless of K, so the old K=32
  top/bot pair paid 2x), with causal column-slicing on diagonal tiles.
- PSUM: s0/s1 are 2-bank strips (2 heads of scoresT each), a0..a3 1-bank AV
  accumulators (with an appended ones-column on V giving the softmax denom).
  Projections use only s0/s1 (QA+QB / KA+KB / V0..V3 share single 1024-wide
  instances), so attention's AV tags never write-after-read-block on the
  projection pipeline.
- Emission order: proj(0), proj(1) up front; then per chunk c:
  attention(c) -> proj(c+2) (PE filler while ACT drains the exp backlog) ->
  normalize(c) -> output-projection(c).
"""
import numpy as np
import ml_dtypes
from contextlib import ExitStack

import concourse.bacc as bacc
import concourse.bass as bass
import concourse.mybir as mybir
import concourse.tile as tile
from concourse._compat import with_exitstack
from concourse.bass_utils import run_bass_kernel_spmd

F32 = mybir.dt.float32
F32R = mybir.dt.float32r
BF16 = mybir.dt.bfloat16

B, L, DM, NH, DK = 2, 2048, 1024, 16, 64
HPC = 4              # heads per core
THETA = 10000.0
CH = 512             # q/l chunk
NT = L // 128        # 16 kv tiles
NCH = L // CH        # 4 chunks

_cache = {}

# stream_shuffle mask: swap 16-blocks within each 32-quadrant (p <-> p^16)
_SWAP16 = list(range(16, 32)) + list(range(16))


@with_exitstack
def _attn_kernel(ctx: ExitStack, tc: tile.TileContext, outs, ins):
    nc = tc.nc
    xt, wq, wv, wo = ins["xt"], ins["wq"], ins["wv"], ins["wo"]
    cs, sn = ins["cs"], ins["sn"]
    out = outs["out"]
    AF = mybir.ActivationFunctionType

    consts = ctx.enter_context(tc.tile_pool(name="consts", bufs=1))
    persist = ctx.enter_context(tc.tile_pool(name="persist", bufs=1))
    ps = ctx.enter_context(tc.tile_pool(name="ps", bufs=1, space="PSUM"))
    xtp = ctx.enter_context(tc.tile_pool(name="xtp", bufs=18))
    csp = ctx.enter_context(tc.tile_pool(name="csp", bufs=3))
    ropet = ctx.enter_context(tc.tile_pool(name="ropet", bufs=2))
    epool = ctx.enter_context(tc.tile_pool(name="epool", bufs=4))
    ipool = ctx.enter_context(tc.tile_pool(name="ipool", bufs=2))
    opool = ctx.enter_context(tc.tile_pool(name="opool", bufs=4))

    # ---- weights / constants ----
    # wq and the first xt chunk are interleaved per-d so the d=0..7
    # accumulation chain starts as soon as each pair lands
    wq_sb = []
    xt0 = []
    for d in range(8):
        t_wq = consts.tile([128, 512], BF16, tag=f"wq{d}")
        nc.sync.dma_start(t_wq, wq[128 * d:128 * d + 128, :])
        wq_sb.append(t_wq)
        t_x0 = xtp.tile([128, CH], BF16, tag="xt")
        nc.sync.dma_start(t_x0, xt[128 * d:128 * d + 128, 0:CH])
        xt0.append(t_x0)
    # lower-triangular keep-mask (keep iff q_local >= kv_local)
    tri = consts.tile([128, 128], BF16)
    nc.vector.memset(tri, 1.0)
    nc.gpsimd.affine_select(tri, tri, pattern=[[1, 128]],
                            compare_op=mybir.AluOpType.is_ge, fill=0.0,
                            base=0, channel_multiplier=-1)

    # persistent activations: head-contiguous RoPE'd Q/K in bf16
    q2a = persist.tile([128, L], BF16)   # heads 0,1
    q2b = persist.tile([128, L], BF16)   # heads 2,3
    k2a = persist.tile([128, L], BF16)
    k2b = persist.tile([128, L], BF16)
    v_sb = []
    for t in range(NT):
        t_v = persist.tile([128, HPC * 65], BF16, tag=f"v{t}")
        v_sb.append(t_v)
        nc.vector.memset(t_v[:, 64:HPC * 65:65], 1.0)
    ho = []
    for j in range(2):
        t_ho = persist.tile([128, L], F32R, tag=f"ho{j}")
        ho.append(t_ho)

    wv_sb = []
    wo_sb = []

    def _load_wv():
        res = []
        for d in range(8):
            t_wv = consts.tile([128, 256], BF16, tag=f"wv{d}")
            nc.sync.dma_start(t_wv, wv[128 * d:128 * d + 128, :])
            res.append(t_wv)
        return res

    def _load_wo():
        res = []
        for j in range(2):
            t_wo = consts.tile([128, DM], F32R, tag=f"wo{j}")
            nc.sync.dma_start(t_wo, wo[128 * j:128 * j + 128, :].bitcast(F32R))
            res.append(t_wo)
        return res

    state = {}

    def emit_xt_dmas(c):
        lsl = slice(CH * c, CH * (c + 1))
        xt_c = []
        for d in range(8):
            t_x = xtp.tile([128, CH], BF16, tag="xt")
            nc.sync.dma_start(t_x, xt[128 * d:128 * d + 128, lsl])
            xt_c.append(t_x)
        cs_c = csp.tile([128, CH], F32, tag="cs")
        nc.sync.dma_start(cs_c, cs[:, lsl])
        sn_c = csp.tile([128, CH], F32, tag="sn")
        nc.sync.dma_start(sn_c, sn[:, lsl])
        state[c] = (xt_c, cs_c, sn_c)

    def emit_rope(psrc, cs_c, sn_c, dst, lsl):
        """RoPE a [128, CH] PSUM pair-tile into bf16 dst[:, lsl]."""
        tmpc = ropet.tile([128, CH], BF16, tag="tc")
        tmps = ropet.tile([128, CH], BF16, tag="ts")
        tmpw = ropet.tile([128, CH], BF16, tag="tw")
        nc.vector.tensor_mul(tmpc, psrc, cs_c)
        nc.vector.tensor_mul(tmps, psrc, sn_c)
        nc.vector.stream_shuffle(tmpw, tmps, mask=_SWAP16)
        # combine on the (otherwise idle) Pool engine: all-SBUF bf16
        nc.gpsimd.tensor_add(dst[:, lsl], tmpc, tmpw)

    def emit_proj(c):
        """QKV projection + RoPE for chunk c (uses only s0/s1 PSUM tags)."""
        xt_c, cs_c, sn_c = state[c]
        lsl = slice(CH * c, CH * (c + 1))
        # Q: pairs A (cols 0:128 of wq) and B (128:256) share one instance
        psq = ps.tile([128, 2 * CH], F32, tag="scP0")
        for d in range(8):
            nc.tensor.matmul(psq[:, 0:CH], wq_sb[d][:, 0:128], xt_c[d][:],
                             start=(d == 0), stop=(d == 7))
            nc.tensor.matmul(psq[:, CH:2 * CH], wq_sb[d][:, 128:256],
                             xt_c[d][:], start=(d == 0), stop=(d == 7))
        emit_rope(psq[:, 0:CH], cs_c, sn_c, q2a, lsl)
        emit_rope(psq[:, CH:2 * CH], cs_c, sn_c, q2b, lsl)
        # K
        psk = ps.tile([128, 2 * CH], F32, tag="scP1")
        for d in range(8):
            nc.tensor.matmul(psk[:, 0:CH], wq_sb[d][:, 256:384], xt_c[d][:],
                             start=(d == 0), stop=(d == 7))
            nc.tensor.matmul(psk[:, CH:2 * CH], wq_sb[d][:, 384:512],
                             xt_c[d][:], start=(d == 0), stop=(d == 7))
        emit_rope(psk[:, 0:CH], cs_c, sn_c, k2a, lsl)
        emit_rope(psk[:, CH:2 * CH], cs_c, sn_c, k2b, lsl)
        # V: 4 l-tiles of [128, 256] share one instance
        psv = ps.tile([128, 2 * CH], F32, tag="scP0")
        for i in range(4):
            t = 4 * c + i
            lo = 128 * i
            for d in range(8):
                nc.tensor.matmul(psv[:, 256 * i:256 * i + 256],
                                 xt_c[d][:, lo:lo + 128], wv_sb[d][:],
                                 start=(d == 0), stop=(d == 7))
        for i in range(4):
            t = 4 * c + i
            vdst = v_sb[t][:].rearrange("p (h x) -> p h x", x=65)[:, :, 0:64]
            vsrc = psv[:, 256 * i:256 * i + 256].rearrange(
                "p (h x) -> p h x", x=64)
            nc.vector.tensor_copy(vdst, vsrc)

    def emit_attention(c):
        """Scores + exp + AV accumulate for chunk c."""
        qsl = slice(CH * c, CH * (c + 1))
        av = []
        for h in range(HPC):
            t_av = ps.tile([65, CH], F32, tag=f"av{h}")
            av.append(t_av)
        ntile = 4 * c + 4
        for t in range(ntile):
            ksl = slice(128 * t, 128 * t + 128)
            diag = (t // 4 == c)
            off = 128 * t - CH * c if diag else 0
            strip0 = ps.tile([128, 2 * CH], F32, tag="scP0")
            strip1 = ps.tile([128, 2 * CH], F32, tag="scP1")
            strips = [strip0, strip1]
            for h in range(HPC):
                k2 = (k2a, k2b)[h // 2]
                q2 = (q2a, q2b)[h // 2]
                hsl = slice(64 * (h % 2), 64 * (h % 2) + 64)
                pss = strips[h // 2][:, CH * (h % 2) + off:CH * (h % 2 + 1)]
                nc.tensor.matmul(pss, k2[hsl, ksl],
                                 q2[hsl, CH * c + off:CH * (c + 1)],
                                 start=True, stop=True,
                                 tile_position=(64 * (h % 2), 0))
            for p in range(2):
                strip = strips[p]
                expt = epool.tile([128, 2 * CH], BF16, tag="expt")
                esrc = strip[:].rearrange("q (h x) -> q h x", x=CH)[:, :, off:]
                edst = expt[:].rearrange("q (h x) -> q h x", x=CH)[:, :, off:]
                nc.scalar.activation(edst, esrc, AF.Exp, scale=0.125)
                if diag:
                    # triangular mask on the 128-wide diagonal block
                    for hh in range(2):
                        blk = slice(CH * hh + off, CH * hh + off + 128)
                        nc.vector.tensor_mul(expt[:, blk], expt[:, blk], tri)
                for hh in range(2):
                    h = 2 * p + hh
                    nc.tensor.matmul(av[h][:, off:],
                                     v_sb[t][:, 65 * h:65 * h + 65],
                                     expt[:, CH * hh + off:CH * (hh + 1)],
                                     start=(t == 0), stop=(t == ntile - 1))
        return av

    def emit_norm_pair(c, av, p):
        """Softmax normalize pair p (heads 2p, 2p+1): ho = av * (1/denom).

        PE-free: reciprocal (DVE) -> partition broadcast (Pool) -> scale
        (DVE); emitted before the next projection so the DVE steps are not
        queued behind its RoPE work.
        """
        qsl = slice(CH * c, CH * (c + 1))
        bcs = []
        for hh in range(2):
            h = 2 * p + hh
            inv = ipool.tile([1, CH], F32R, tag=f"inv{h}")
            with nc.allow_low_precision(reason="tf32 softmax denom"):
                nc.vector.reciprocal(inv, av[h][64:65, :])
            bch = ipool.tile([64, CH], F32R, tag=f"bc{h}")
            nc.gpsimd.partition_broadcast(bch, inv)
            bcs.append(bch)
        for hh in range(2):
            h = 2 * p + hh
            nc.vector.tensor_mul(
                ho[h // 2][64 * (h % 2):64 * (h % 2) + 64, qsl],
                av[h][0:64, :], bcs[hh])

    def emit_outproj(c, j, tiles):
        """One accumulation round (j) of the output projection of chunk c.

        tiles: list that is filled with the ps_o tiles on round 0 and
        reused (stop + copy out) on round 1.
        """
        for i, lt in enumerate(range(4 * c, 4 * c + 4)):
            for oc in range(2):
                osl = slice(512 * oc, 512 * oc + 512)
                if j == 0:
                    ps_o = ps.tile([128, 512], F32,
                                   tag=f"av{(2 * lt + oc) % 4}", name="ps_o")
                    tiles.append(ps_o)
                ps_o = tiles[2 * i + oc]
                nc.tensor.matmul(ps_o, ho[j][:, 128 * lt:128 * lt + 128],
                                 wo_sb[j][:, osl],
                                 start=(j == 0), stop=(j == 1))
                if j == 1:
                    # stage on ACT: idle in this window (exp backlog drained)
                    o_sb = opool.tile([128, 512], F32, tag="o")
                    nc.scalar.copy(o_sb, ps_o)
                    nc.sync.dma_start(out[128 * lt:128 * lt + 128, osl], o_sb)

    # ---- software pipeline ----
    cs_0 = csp.tile([128, CH], F32, tag="cs")
    nc.sync.dma_start(cs_0, cs[:, 0:CH])
    sn_0 = csp.tile([128, CH], F32, tag="sn")
    nc.sync.dma_start(sn_0, sn[:, 0:CH])
    state[0] = (xt0, cs_0, sn_0)
    wv_sb.extend(_load_wv())
    emit_xt_dmas(1)
    emit_proj(0)
    wo_sb.extend(_load_wo())
    emit_xt_dmas(2)
    emit_proj(1)
    for c in range(NCH):
        av = emit_attention(c)
        tiles = []
        if c + 2 < NCH:
            # projection PE work covers the whole normalize chain
            emit_norm_pair(c, av, 0)
            emit_norm_pair(c, av, 1)
            if c + 3 < NCH:
                emit_xt_dmas(c + 3)
            emit_proj(c + 2)
            emit_outproj(c, 0, tiles)
            emit_outproj(c, 1, tiles)
        else:
            # no projection filler left: split per pair so outproj round 0
            # starts as soon as pair A is normalized
            emit_norm_pair(c, av, 0)
            emit_outproj(c, 0, tiles)
            emit_norm_pair(c, av, 1)
            emit_outproj(c, 1, tiles)


def _build_nc():
    nc = bacc.Bacc("TRN2", target_bir_lowering=False, debug=False,
                   enable_asserts=False, num_devices=8)
    ins = {
        "xt": nc.dram_tensor("xt", [DM, L], BF16, kind="ExternalInput").ap(),
        "wq": nc.dram_tensor("wq", [DM, 512], BF16, kind="ExternalInput").ap(),
        "wv": nc.dram_tensor("wv", [DM, 256], BF16, kind="ExternalInput").ap(),
        "wo": nc.dram_tensor("wo", [256, DM], F32, kind="ExternalInput").ap(),
        "cs": nc.dram_tensor("cs", [128, L], F32, kind="ExternalInput").ap(),
        "sn": nc.dram_tensor("sn", [128, L], F32, kind="ExternalInput").ap(),
    }
    outs = {"out": nc.dram_tensor("out", [L, DM], F32, kind="ExternalOutput").ap()}
    with tile.TileContext(nc) as tc:
        _attn_kernel(tc, outs, ins)
    nc.compile()
    return nc


def _host_shard(X, token_positions, Wqkv, Wout):
    """Build the 8 per-core input maps."""
    X = np.asarray(X, dtype=np.float32)
    Wqkv = np.asarray(Wqkv, dtype=np.float32)
    Wout = np.asarray(Wout, dtype=np.float32)
    pos = np.asarray(token_positions)

    # Per-head dim order: [e0..e15, o0..o15, e16..e31, o16..o31] so the RoPE
    # partner (even<->odd of the same freq) is p^16 within a 32-quadrant.
    def head_order(base):
        return ([base + 2 * k for k in range(16)] +
                [base + 2 * k + 1 for k in range(16)] +
                [base + 2 * k for k in range(16, 32)] +
                [base + 2 * k + 1 for k in range(16, 32)])

    # RoPE tables in float32 arithmetic to mirror the f32 reference
    k = np.arange(DK // 2, dtype=np.float32)
    inv_freq = (np.float32(1.0) /
                np.power(np.float32(THETA), (np.float32(2.0) * k) / np.float32(DK)))
    inv_freq = inv_freq.astype(np.float32)
    # freq index per partition within a 64-block: [0:16]=f0..15, [16:32]=f0..15,
    # [32:48]=f16..31, [48:64]=f16..31; sign +1 on top blocks, -1 on bot blocks
    fidx = np.concatenate([np.arange(16), np.arange(16),
                           np.arange(16, 32), np.arange(16, 32)])
    sgn = np.concatenate([np.ones(16), -np.ones(16),
                          np.ones(16), -np.ones(16)]).astype(np.float32)
    fidx = np.tile(fidx, 2)           # 128 partitions (2 heads per tile)
    sgn = np.tile(sgn, 2)
    ang = (pos.astype(np.float32)[:, None, :] *
           inv_freq[fidx][None, :, None]).astype(np.float32)   # [B, 128, L]
    cs_all = np.cos(ang).astype(np.float32)
    sn_all = (np.sin(ang) * sgn[None, :, None]).astype(np.float32)

    in_maps = []
    for core in range(8):
        b, g = divmod(core, HPC)
        heads = [HPC * g + hh for hh in range(HPC)]
        q_rows, k_rows = [], []
        for h in heads:
            q_rows += head_order(DK * h)
            k_rows += head_order(DM + DK * h)
        wq_c = np.ascontiguousarray(Wqkv[q_rows + k_rows, :].T
                                    .astype(ml_dtypes.bfloat16))
        v_rows = [2 * DM + DK * h + j for h in heads for j in range(DK)]
        wv_c = np.ascontiguousarray(Wqkv[v_rows, :].T.astype(ml_dtypes.bfloat16))
        wo_c = np.ascontiguousarray(Wout[:, 256 * g:256 * (g + 1)].T)
        in_maps.append({
            "xt": np.ascontiguousarray(X[b].T.astype(ml_dtypes.bfloat16)),
            "wq": wq_c,
            "wv": wv_c,
            "wo": wo_c,
            "cs": np.ascontiguousarray(cs_all[b]),
            "sn": np.ascontiguousarray(sn_all[b]),
        })
    return in_maps


def kernel(X, token_positions, Wqkv, Wout, _trace=False):
    if "nc" not in _cache:
        _cache["nc"] = _build_nc()
    nc = _cache["nc"]
    in_maps = _host_shard(X, token_positions, Wqkv, Wout)
    res = run_bass_kernel_spmd(nc, in_maps, list(range(8)), trace=_trace)
    _cache["last_results"] = res
    out = np.zeros((B, L, DM), dtype=np.float32)
    for core in range(8):
        out[core // HPC] += res.results[core]["out"]
    return out


# revision 24
# speedup vs baseline: 1.2156x; 1.0699x over previous
"""Causal multi-head attention with RoPE on 8 Trainium2 NeuronCores.

Problem: B=2, L=2048, D_MODEL=1024, N_HEADS=16, D_K=64, theta=10000.
Sharding: data parallel on batch (2) x tensor parallel on heads (4 groups of
4 heads) = 8 cores. Each core computes its 4 heads' attention plus a partial
output projection; partials are summed on the host (Megatron row-parallel).

Per-core device design (v3):
- Q/K live head-contiguous in bf16: each head owns 64 partitions, with its
  RoPE top (even) / bot (odd) dims interleaved in 16-blocks so the rotation
  partner is always p^16 within a 32-quadrant.  RoPE is then 2 full-width
  DVE muls (cos / sign-folded sin) + one stream_shuffle(+-16) + one bf16
  add.  Scores become a SINGLE K=64 matmul per head-tile (the cost model
  charges a matmul by its moving size regardless of K, so the old K=32
  top/bot pair paid 2x), with causal column-slicing on diagonal tiles.
- PSUM: s0/s1 are 2-bank strips (2 heads of scoresT each), a0..a3 1-bank AV
  accumulators (with an appended ones-column on V giving the softmax denom).
  Projections use only s0/s1 (QA+QB / KA+KB / V0..V3 share single 1024-wide
  instances), so attention's AV tags never write-after-read-block on the
  projection pipeline.
- Emission order: proj(0), proj(1) up front; then per chunk c:
  attention(c) -> proj(c+2) (PE filler while ACT drains the exp backlog) ->
  normalize(c) -> output-projection(c).
"""
import numpy as np
import ml_dtypes
from contextlib import ExitStack

import concourse.bacc as bacc
import concourse.bass as bass
import concourse.mybir as mybir
import concourse.tile as tile
from concourse._compat import with_exitstack
from concourse.bass_utils import run_bass_kernel_spmd

F32 = mybir.dt.float32
F32R = mybir.dt.float32r
BF16 = mybir.dt.bfloat16

B, L, DM, NH, DK = 2, 2048, 1024, 16, 64
HPC = 4              # heads per core
THETA = 10000.0
CH = 512             # q/l chunk
NT = L // 128        # 16 kv tiles
NCH = L // CH        # 4 chunks

_cache = {}

# stream_shuffle mask: swap 16-blocks within each 32-quadrant (p <-> p^16)
_SWAP16 = list(range(16, 32)) + list(range(16))


@with_exitstack
def _attn_kernel(ctx: ExitStack, tc: tile.TileContext, outs, ins):
    nc = tc.nc
    xt, wq, wv, wo = ins["xt"], ins["wq"], ins["wv"], ins["wo"]
    cs, sn = ins["cs"], ins["sn"]
    out = outs["out"]
    AF = mybir.ActivationFunctionType

    consts = ctx.enter_context(tc.tile_pool(name="consts", bufs=1))
    persist = ctx.enter_context(tc.tile_pool(name="persist", bufs=1))
    ps = ctx.enter_context(tc.tile_pool(name="ps", bufs=1, space="PSUM"))
    xtp = ctx.enter_context(tc.tile_pool(name="xtp", bufs=18))
    csp = ctx.enter_context(tc.tile_pool(name="csp", bufs=3))
    ropet = ctx.enter_context(tc.tile_pool(name="ropet", bufs=2))
    epool = ctx.enter_context(tc.tile_pool(name="epool", bufs=10))
    ipool = ctx.enter_context(tc.tile_pool(name="ipool", bufs=2))
    opool = ctx.enter_context(tc.tile_pool(name="opool", bufs=4))

    # ---- weights / constants ----
    # wq and the first xt chunk are interleaved per-d so the d=0..7
    # accumulation chain starts as soon as each pair lands
    wq_sb = []
    xt0 = []
    for d in range(8):
        t_wq = consts.tile([128, 512], BF16, tag=f"wq{d}")
        nc.sync.dma_start(t_wq, wq[128 * d:128 * d + 128, :])
        wq_sb.append(t_wq)
        t_x0 = xtp.tile([128, CH], BF16, tag="xt")
        nc.sync.dma_start(t_x0, xt[128 * d:128 * d + 128, 0:CH])
        xt0.append(t_x0)
    # lower-triangular keep-mask (keep iff q_local >= kv_local)
    tri = consts.tile([128, 128], BF16)
    nc.vector.memset(tri, 1.0)
    nc.gpsimd.affine_select(tri, tri, pattern=[[1, 128]],
                            compare_op=mybir.AluOpType.is_ge, fill=0.0,
                            base=0, channel_multiplier=-1)

    # persistent activations: head-contiguous RoPE'd Q/K in bf16
    q2a = persist.tile([128, L], BF16)   # heads 0,1
    q2b = persist.tile([128, L], BF16)   # heads 2,3
    k2a = persist.tile([128, L], BF16)
    k2b = persist.tile([128, L], BF16)
    v_sb = []
    for t in range(NT):
        t_v = persist.tile([128, HPC * 65], BF16, tag=f"v{t}")
        v_sb.append(t_v)
        nc.vector.memset(t_v[:, 64:HPC * 65:65], 1.0)
    ho = []
    for j in range(2):
        t_ho = persist.tile([128, L], F32R, tag=f"ho{j}")
        ho.append(t_ho)

    wv_sb = []
    wo_sb = []

    def _load_wv():
        res = []
        for d in range(8):
            t_wv = consts.tile([128, 256], BF16, tag=f"wv{d}")
            nc.sync.dma_start(t_wv, wv[128 * d:128 * d + 128, :])
            res.append(t_wv)
        return res

    def _load_wo():
        res = []
        for j in range(2):
            t_wo = consts.tile([128, DM], F32R, tag=f"wo{j}")
            nc.sync.dma_start(t_wo, wo[128 * j:128 * j + 128, :].bitcast(F32R))
            res.append(t_wo)
        return res

    state = {}

    def emit_xt_dmas(c):
        lsl = slice(CH * c, CH * (c + 1))
        xt_c = []
        for d in range(8):
            t_x = xtp.tile([128, CH], BF16, tag="xt")
            nc.sync.dma_start(t_x, xt[128 * d:128 * d + 128, lsl])
            xt_c.append(t_x)
        cs_c = csp.tile([128, CH], F32, tag="cs")
        nc.sync.dma_start(cs_c, cs[:, lsl])
        sn_c = csp.tile([128, CH], F32, tag="sn")
        nc.sync.dma_start(sn_c, sn[:, lsl])
        state[c] = (xt_c, cs_c, sn_c)

    def rope_muls(psrc, cs_c, sn_c):
        """cos/sin products of a [128, CH] PSUM pair-tile (releases psrc)."""
        tmpc = ropet.tile([128, CH], BF16, tag="tc")
        tmps = ropet.tile([128, CH], BF16, tag="ts")
        nc.vector.tensor_mul(tmpc, psrc, cs_c)
        nc.vector.tensor_mul(tmps, psrc, sn_c)
        return tmpc, tmps

    def rope_combine(tmpc, tmps, dst, lsl):
        tmpw = ropet.tile([128, CH], BF16, tag="tw")
        nc.vector.stream_shuffle(tmpw, tmps, mask=_SWAP16)
        # combine on the (otherwise idle) Pool engine: all-SBUF bf16
        nc.gpsimd.tensor_add(dst[:, lsl], tmpc, tmpw)

    def emit_proj_q(c):
        """Q projection + RoPE for chunk c (s0 PSUM tag)."""
        xt_c, cs_c, sn_c = state[c]
        lsl = slice(CH * c, CH * (c + 1))
        # pairs A (cols 0:128 of wq) and B (128:256) share one instance
        psq = ps.tile([128, 2 * CH], F32, tag="scP0")
        for d in range(8):
            nc.tensor.matmul(psq[:, 0:CH], wq_sb[d][:, 0:128], xt_c[d][:],
                             start=(d == 0), stop=(d == 7))
            nc.tensor.matmul(psq[:, CH:2 * CH], wq_sb[d][:, 128:256],
                             xt_c[d][:], start=(d == 0), stop=(d == 7))
        ca = rope_muls(psq[:, 0:CH], cs_c, sn_c)
        cb = rope_muls(psq[:, CH:2 * CH], cs_c, sn_c)
        rope_combine(*ca, q2a, lsl)
        rope_combine(*cb, q2b, lsl)

    def emit_proj_kv(c):
        """K+V projections for chunk c; DVE order: K muls, V copies, then
        K shuffles — so the strip tags are released as early as possible."""
        xt_c, cs_c, sn_c = state[c]
        lsl = slice(CH * c, CH * (c + 1))
        psk = ps.tile([128, 2 * CH], F32, tag="scP1")
        for d in range(8):
            nc.tensor.matmul(psk[:, 0:CH], wq_sb[d][:, 256:384], xt_c[d][:],
                             start=(d == 0), stop=(d == 7))
            nc.tensor.matmul(psk[:, CH:2 * CH], wq_sb[d][:, 384:512],
                             xt_c[d][:], start=(d == 0), stop=(d == 7))
        # V: 4 l-tiles of [128, 256] share one instance
        psv = ps.tile([128, 2 * CH], F32, tag="scP0")
        for i in range(4):
            lo = 128 * i
            for d in range(8):
                nc.tensor.matmul(psv[:, 256 * i:256 * i + 256],
                                 xt_c[d][:, lo:lo + 128], wv_sb[d][:],
                                 start=(d == 0), stop=(d == 7))
        ca = rope_muls(psk[:, 0:CH], cs_c, sn_c)
        cb = rope_muls(psk[:, CH:2 * CH], cs_c, sn_c)
        for i in range(4):
            t = 4 * c + i
            vdst = v_sb[t][:].rearrange("p (h x) -> p h x", x=65)[:, :, 0:64]
            vsrc = psv[:, 256 * i:256 * i + 256].rearrange(
                "p (h x) -> p h x", x=64)
            nc.vector.tensor_copy(vdst, vsrc)
        rope_combine(*ca, k2a, lsl)
        rope_combine(*cb, k2b, lsl)

    def emit_scores_tile(c, t):
        """Scores + exp (+ causal mask) for kv tile t of chunk c.
        Returns the two bf16 expt tiles and the causal column offset."""
        ksl = slice(128 * t, 128 * t + 128)
        diag = (t // 4 == c)
        off = 128 * t - CH * c if diag else 0
        strip0 = ps.tile([128, 2 * CH], F32, tag="scP0")
        strip1 = ps.tile([128, 2 * CH], F32, tag="scP1")
        strips = [strip0, strip1]
        for h in range(HPC):
            k2 = (k2a, k2b)[h // 2]
            q2 = (q2a, q2b)[h // 2]
            hsl = slice(64 * (h % 2), 64 * (h % 2) + 64)
            pss = strips[h // 2][:, CH * (h % 2) + off:CH * (h % 2 + 1)]
            nc.tensor.matmul(pss, k2[hsl, ksl],
                             q2[hsl, CH * c + off:CH * (c + 1)],
                             start=True, stop=True,
                             tile_position=(64 * (h % 2), 0))
        expts = []
        for p in range(2):
            strip = strips[p]
            expt = epool.tile([128, 2 * CH], BF16, tag="expt")
            esrc = strip[:].rearrange("q (h x) -> q h x", x=CH)[:, :, off:]
            edst = expt[:].rearrange("q (h x) -> q h x", x=CH)[:, :, off:]
            nc.scalar.activation(edst, esrc, AF.Exp, scale=0.125)
            if diag:
                # triangular mask on the 128-wide diagonal block
                for hh in range(2):
                    blk = slice(CH * hh + off, CH * hh + off + 128)
                    nc.vector.tensor_mul(expt[:, blk], expt[:, blk], tri)
            expts.append(expt)
        return expts, off

    def emit_av_tile(c, t, av, expts, off):
        ntile = 4 * c + 4
        for p in range(2):
            for hh in range(2):
                h = 2 * p + hh
                nc.tensor.matmul(av[h][:, off:],
                                 v_sb[t][:, 65 * h:65 * h + 65],
                                 expts[p][:, CH * hh + off:CH * (hh + 1)],
                                 start=(t == 0), stop=(t == ntile - 1))

    def alloc_av():
        av = []
        for h in range(HPC):
            t_av = ps.tile([65, CH], F32, tag=f"av{h}", name="t_av")
            av.append(t_av)
        return av

    def emit_attention(c, av, t0=0, pre=()):
        """AV for prefetched tiles `pre`, then full tiles t0..ntile-1."""
        for t, (expts, off) in enumerate(pre):
            emit_av_tile(c, t, av, expts, off)
        for t in range(t0, 4 * c + 4):
            expts, off = emit_scores_tile(c, t)
            emit_av_tile(c, t, av, expts, off)

    def emit_norm_pair(c, av, p):
        """Softmax normalize pair p (heads 2p, 2p+1): ho = av * (1/denom).

        PE-free: reciprocal (DVE) -> partition broadcast (Pool) -> scale
        (DVE); emitted before the next projection so the DVE steps are not
        queued behind its RoPE work.
        """
        qsl = slice(CH * c, CH * (c + 1))
        bcs = []
        for hh in range(2):
            h = 2 * p + hh
            inv = ipool.tile([1, CH], F32R, tag=f"inv{h}")
            with nc.allow_low_precision(reason="tf32 softmax denom"):
                nc.vector.reciprocal(inv, av[h][64:65, :])
            bch = ipool.tile([64, CH], F32R, tag=f"bc{h}")
            nc.gpsimd.partition_broadcast(bch, inv)
            bcs.append(bch)
        for hh in range(2):
            h = 2 * p + hh
            nc.vector.tensor_mul(
                ho[h // 2][64 * (h % 2):64 * (h % 2) + 64, qsl],
                av[h][0:64, :], bcs[hh])

    def emit_outproj_group(c, g):
        """Half of chunk c's output projection (2 l-tiles x 2 column halves)
        on av0..av3 — each tag used once per group, j rounds interleaved,
        staging copies alternating DVE/ACT."""
        tiles = []
        for i, lt in enumerate((4 * c + 2 * g, 4 * c + 2 * g + 1)):
            for oc in range(2):
                ps_o = ps.tile([128, 512], F32, tag=f"av{2 * i + oc}",
                               name="ps_o")
                tiles.append((ps_o, lt, oc))
        for j in range(2):
            for ps_o, lt, oc in tiles:
                osl = slice(512 * oc, 512 * oc + 512)
                nc.tensor.matmul(ps_o, ho[j][:, 128 * lt:128 * lt + 128],
                                 wo_sb[j][:, osl],
                                 start=(j == 0), stop=(j == 1))
        for n, (ps_o, lt, oc) in enumerate(tiles):
            osl = slice(512 * oc, 512 * oc + 512)
            o_sb = opool.tile([128, 512], F32, tag="o")
            if n % 2 == 0:
                nc.vector.tensor_copy(o_sb, ps_o)
            else:
                nc.scalar.copy(o_sb, ps_o)
            nc.sync.dma_start(out[128 * lt:128 * lt + 128, osl], o_sb)

    # ---- software pipeline ----
    cs_0 = csp.tile([128, CH], F32, tag="cs")
    nc.sync.dma_start(cs_0, cs[:, 0:CH])
    sn_0 = csp.tile([128, CH], F32, tag="sn")
    nc.sync.dma_start(sn_0, sn[:, 0:CH])
    state[0] = (xt0, cs_0, sn_0)
    wv_sb.extend(_load_wv())
    emit_xt_dmas(1)
    emit_proj_q(0)
    emit_proj_kv(0)
    wo_sb.extend(_load_wo())
    emit_xt_dmas(2)
    emit_proj_q(1)
    emit_proj_kv(1)
    for c in range(2):
        av = alloc_av()
        emit_attention(c, av)
        # Q-projection PE work first so its RoPE products lead the DVE
        # queue; the normalize chain then hides under the K/V projections
        emit_proj_q(c + 2)
        emit_norm_pair(c, av, 0)
        emit_norm_pair(c, av, 1)
        if c + 3 < NCH:
            emit_xt_dmas(c + 3)
        emit_proj_kv(c + 2)
        emit_outproj_group(c, 0)
        emit_outproj_group(c, 1)
    # c = 2: no projection filler left — prefetch the first scores of
    # chunk 3 (deferring their AV matmuls) to keep PE fed during norm(2)
    av = alloc_av()
    emit_attention(2, av)
    pre = [emit_scores_tile(3, t) for t in range(4)]
    emit_norm_pair(2, av, 0)
    emit_norm_pair(2, av, 1)
    emit_outproj_group(2, 0)
    emit_outproj_group(2, 1)
    # c = 3
    av = alloc_av()
    emit_attention(3, av, t0=4, pre=pre)
    emit_norm_pair(3, av, 0)
    emit_norm_pair(3, av, 1)
    emit_outproj_group(3, 0)
    emit_outproj_group(3, 1)


def _build_nc():
    nc = bacc.Bacc("TRN2", target_bir_lowering=False, debug=False,
                   enable_asserts=False, num_devices=8)
    ins = {
        "xt": nc.dram_tensor("xt", [DM, L], BF16, kind="ExternalInput").ap(),
        "wq": nc.dram_tensor("wq", [DM, 512], BF16, kind="ExternalInput").ap(),
        "wv": nc.dram_tensor("wv", [DM, 256], BF16, kind="ExternalInput").ap(),
        "wo": nc.dram_tensor("wo", [256, DM], F32, kind="ExternalInput").ap(),
        "cs": nc.dram_tensor("cs", [128, L], F32, kind="ExternalInput").ap(),
        "sn": nc.dram_tensor("sn", [128, L], F32, kind="ExternalInput").ap(),
    }
    outs = {"out": nc.dram_tensor("out", [L, DM], F32, kind="ExternalOutput").ap()}
    with tile.TileContext(nc) as tc:
        _attn_kernel(tc, outs, ins)
    nc.compile()
    return nc


def _host_shard(X, token_positions, Wqkv, Wout):
    """Build the 8 per-core input maps."""
    X = np.asarray(X, dtype=np.float32)
    Wqkv = np.asarray(Wqkv, dtype=np.float32)
    Wout = np.asarray(Wout, dtype=np.float32)
    pos = np.asarray(token_positions)

    # Per-head dim order: [e0..e15, o0..o15, e16..e31, o16..o31] so the RoPE
    # partner (even<->odd of the same freq) is p^16 within a 32-quadrant.
    def head_order(base):
        return ([base + 2 * k for k in range(16)] +
                [base + 2 * k + 1 for k in range(16)] +
                [base + 2 * k for k in range(16, 32)] +
                [base + 2 * k + 1 for k in range(16, 32)])

    # RoPE tables in float32 arithmetic to mirror the f32 reference
    k = np.arange(DK // 2, dtype=np.float32)
    inv_freq = (np.float32(1.0) /
                np.power(np.float32(THETA), (np.float32(2.0) * k) / np.float32(DK)))
    inv_freq = inv_freq.astype(np.float32)
    # freq index per partition within a 64-block: [0:16]=f0..15, [16:32]=f0..15,
    # [32:48]=f16..31, [48:64]=f16..31; sign +1 on top blocks, -1 on bot blocks
    fidx = np.concatenate([np.arange(16), np.arange(16),
                           np.arange(16, 32), np.arange(16, 32)])
    sgn = np.concatenate([np.ones(16), -np.ones(16),
                          np.ones(16), -np.ones(16)]).astype(np.float32)
    fidx = np.tile(fidx, 2)           # 128 partitions (2 heads per tile)
    sgn = np.tile(sgn, 2)
    ang = (pos.astype(np.float32)[:, None, :] *
           inv_freq[fidx][None, :, None]).astype(np.float32)   # [B, 128, L]
    cs_all = np.cos(ang).astype(np.float32)
    sn_all = (np.sin(ang) * sgn[None, :, None]).astype(np.float32)

    in_maps = []
    for core in range(8):
        b, g = divmod(core, HPC)
        heads = [HPC * g + hh for hh in range(HPC)]
        q_rows, k_rows = [], []
        for h in heads:
            q_rows += head_order(DK * h)
            k_rows += head_order(DM + DK * h)
        wq_c = np.ascontiguousarray(Wqkv[q_rows + k_rows, :].T
                                    .astype(ml_dtypes.bfloat16))
        v_rows = [2 * DM + DK * h + j for h in heads for j in range(DK)]
        wv_c = np.ascontiguousarray(Wqkv[v_rows, :].T.astype(ml_dtypes.bfloat16))
        wo_c = np.ascontiguousarray(Wout[:, 256 * g:256 * (g + 1)].T)
        in_maps.append({
            "xt": np.ascontiguousarray(X[b].T.astype(ml_dtypes.bfloat16)),
            "wq": wq_c,
            "wv": wv_c,
            "wo": wo_c,
            "cs": np.ascontiguousarray(cs_all[b]),
            "sn": np.ascontiguousarray(sn_all[b]),
        })
    return in_maps


def kernel(X, token_positions, Wqkv, Wout, _trace=False):
    if "nc" not in _cache:
        _cache["nc"] = _build_nc()
    nc = _cache["nc"]
    in_maps = _host_shard(X, token_positions, Wqkv, Wout)
    res = run_bass_kernel_spmd(nc, in_maps, list(range(8)), trace=_trace)
    _cache["last_results"] = res
    out = np.zeros((B, L, DM), dtype=np.float32)
    for core in range(8):
        out[core // HPC] += res.results[core]["out"]
    return out


# revision 32
# speedup vs baseline: 1.2167x; 1.0009x over previous
"""Causal multi-head attention with RoPE on 8 Trainium2 NeuronCores.

Problem: B=2, L=2048, D_MODEL=1024, N_HEADS=16, D_K=64, theta=10000.
Sharding: data parallel on batch (2) x tensor parallel on heads (4 groups of
4 heads) = 8 cores. Each core computes its 4 heads' attention plus a partial
output projection; partials are summed on the host (Megatron row-parallel).

Per-core device design (v3):
- Q/K live head-contiguous in bf16: each head owns 64 partitions, with its
  RoPE top (even) / bot (odd) dims interleaved in 16-blocks so the rotation
  partner is always p^16 within a 32-quadrant.  RoPE is then 2 full-width
  DVE muls (cos / sign-folded sin) + one stream_shuffle(+-16) + one bf16
  add.  Scores become a SINGLE K=64 matmul per head-tile (the cost model
  charges a matmul by its moving size regardless of K, so the old K=32
  top/bot pair paid 2x), with causal column-slicing on diagonal tiles.
- PSUM: s0/s1 are 2-bank strips (2 heads of scoresT each), a0..a3 1-bank AV
  accumulators (with an appended ones-column on V giving the softmax denom).
  Projections use only s0/s1 (QA+QB / KA+KB / V0..V3 share single 1024-wide
  instances), so attention's AV tags never write-after-read-block on the
  projection pipeline.
- Emission order: proj(0), proj(1) up front; then per chunk c:
  attention(c) -> proj(c+2) (PE filler while ACT drains the exp backlog) ->
  normalize(c) -> output-projection(c).
"""
import numpy as np
import ml_dtypes
from contextlib import ExitStack

import concourse.bacc as bacc
import concourse.bass as bass
import concourse.mybir as mybir
import concourse.tile as tile
from concourse._compat import with_exitstack
from concourse.bass_utils import run_bass_kernel_spmd

F32 = mybir.dt.float32
F32R = mybir.dt.float32r
BF16 = mybir.dt.bfloat16

B, L, DM, NH, DK = 2, 2048, 1024, 16, 64
HPC = 4              # heads per core
THETA = 10000.0
CH = 512             # q/l chunk
NT = L // 128        # 16 kv tiles
NCH = L // CH        # 4 chunks

_cache = {}

# stream_shuffle mask: swap 16-blocks within each 32-quadrant (p <-> p^16)
_SWAP16 = list(range(16, 32)) + list(range(16))


@with_exitstack
def _attn_kernel(ctx: ExitStack, tc: tile.TileContext, outs, ins):
    nc = tc.nc
    xt, wq, wv, wo = ins["xt"], ins["wq"], ins["wv"], ins["wo"]
    cs, sn = ins["cs"], ins["sn"]
    out = outs["out"]
    AF = mybir.ActivationFunctionType

    consts = ctx.enter_context(tc.tile_pool(name="consts", bufs=1))
    persist = ctx.enter_context(tc.tile_pool(name="persist", bufs=1))
    ps = ctx.enter_context(tc.tile_pool(name="ps", bufs=1, space="PSUM"))
    ropet = ctx.enter_context(tc.tile_pool(name="ropet", bufs=2))
    epool = ctx.enter_context(tc.tile_pool(name="epool", bufs=10))
    ipool = ctx.enter_context(tc.tile_pool(name="ipool", bufs=2))
    opool = ctx.enter_context(tc.tile_pool(name="opool", bufs=4))

    # ---- weights / inputs ----
    # xt is loaded as full [128, L] rows: one descriptor per partition for
    # the whole row keeps the (descriptor-bound) HWDGE cost per byte low.
    # wq and xt are interleaved per-d so the d-chain starts early.
    wq_sb = []
    xt_sb = []
    for d in range(8):
        t_wq = consts.tile([128, 512], BF16, tag=f"wq{d}")
        nc.sync.dma_start(t_wq, wq[128 * d:128 * d + 128, :])
        wq_sb.append(t_wq)
        t_x = consts.tile([128, L], BF16, tag=f"xt{d}")
        nc.sync.dma_start(t_x, xt[128 * d:128 * d + 128, :])
        xt_sb.append(t_x)
    # RoPE tables: one full-row DMA each (descriptor-efficient)
    cs_sb = consts.tile([128, L], F32)
    nc.sync.dma_start(cs_sb, cs[:, :])
    sn_sb = consts.tile([128, L], F32)
    nc.sync.dma_start(sn_sb, sn[:, :])
    # lower-triangular keep-mask (keep iff q_local >= kv_local)
    tri = consts.tile([128, 128], BF16)
    nc.vector.memset(tri, 1.0)
    nc.gpsimd.affine_select(tri, tri, pattern=[[1, 128]],
                            compare_op=mybir.AluOpType.is_ge, fill=0.0,
                            base=0, channel_multiplier=-1)

    # persistent activations: head-contiguous RoPE'd Q/K in bf16
    q2a = persist.tile([128, L], BF16)   # heads 0,1
    q2b = persist.tile([128, L], BF16)   # heads 2,3
    k2a = persist.tile([128, L], BF16)
    k2b = persist.tile([128, L], BF16)
    v_sb = []
    for t in range(NT):
        t_v = persist.tile([128, HPC * 65], BF16, tag=f"v{t}")
        v_sb.append(t_v)
        nc.vector.memset(t_v[:, 64:HPC * 65:65], 1.0)
    ho = []
    for j in range(2):
        t_ho = persist.tile([128, L], F32R, tag=f"ho{j}")
        ho.append(t_ho)

    wv_sb = []
    wo_sb = []

    def _load_wv():
        res = []
        for d in range(8):
            t_wv = consts.tile([128, 256], BF16, tag=f"wv{d}")
            nc.sync.dma_start(t_wv, wv[128 * d:128 * d + 128, :])
            res.append(t_wv)
        return res

    def _load_wo():
        res = []
        for j in range(2):
            t_wo = consts.tile([128, DM], F32R, tag=f"wo{j}")
            nc.sync.dma_start(t_wo, wo[128 * j:128 * j + 128, :].bitcast(F32R))
            res.append(t_wo)
        return res

    def rope_muls(psrc, cs_c, sn_c):
        """cos/sin products of a [128, CH] PSUM pair-tile (releases psrc)."""
        tmpc = ropet.tile([128, CH], BF16, tag="tc")
        tmps = ropet.tile([128, CH], BF16, tag="ts")
        nc.vector.tensor_mul(tmpc, psrc, cs_c)
        nc.vector.tensor_mul(tmps, psrc, sn_c)
        return tmpc, tmps

    def rope_combine(tmpc, tmps, dst, lsl):
        tmpw = ropet.tile([128, CH], BF16, tag="tw")
        nc.vector.stream_shuffle(tmpw, tmps, mask=_SWAP16)
        # combine on the (otherwise idle) Pool engine: all-SBUF bf16
        nc.gpsimd.tensor_add(dst[:, lsl], tmpc, tmpw)

    def emit_proj_q(c):
        """Q projection + RoPE for chunk c (s0 PSUM tag)."""
        lsl = slice(CH * c, CH * (c + 1))
        cs_c, sn_c = cs_sb[:, lsl], sn_sb[:, lsl]
        # pairs A (cols 0:128 of wq) and B (128:256) share one instance
        psq = ps.tile([128, 2 * CH], F32, tag="scP0")
        for d in range(8):
            nc.tensor.matmul(psq[:, 0:CH], wq_sb[d][:, 0:128],
                             xt_sb[d][:, lsl], start=(d == 0), stop=(d == 7))
            nc.tensor.matmul(psq[:, CH:2 * CH], wq_sb[d][:, 128:256],
                             xt_sb[d][:, lsl], start=(d == 0), stop=(d == 7))
        ca = rope_muls(psq[:, 0:CH], cs_c, sn_c)
        cb = rope_muls(psq[:, CH:2 * CH], cs_c, sn_c)
        rope_combine(*ca, q2a, lsl)
        rope_combine(*cb, q2b, lsl)

    def emit_proj_kv(c):
        """K+V projections for chunk c; DVE order: K muls, V copies, then
        K shuffles — so the strip tags are released as early as possible."""
        lsl = slice(CH * c, CH * (c + 1))
        cs_c, sn_c = cs_sb[:, lsl], sn_sb[:, lsl]
        psk = ps.tile([128, 2 * CH], F32, tag="scP1")
        for d in range(8):
            nc.tensor.matmul(psk[:, 0:CH], wq_sb[d][:, 256:384],
                             xt_sb[d][:, lsl], start=(d == 0), stop=(d == 7))
            nc.tensor.matmul(psk[:, CH:2 * CH], wq_sb[d][:, 384:512],
                             xt_sb[d][:, lsl], start=(d == 0), stop=(d == 7))
        # V: 4 l-tiles of [128, 256] share one instance
        psv = ps.tile([128, 2 * CH], F32, tag="scP0")
        for i in range(4):
            lo = CH * c + 128 * i
            for d in range(8):
                nc.tensor.matmul(psv[:, 256 * i:256 * i + 256],
                                 xt_sb[d][:, lo:lo + 128], wv_sb[d][:],
                                 start=(d == 0), stop=(d == 7))
        ca = rope_muls(psk[:, 0:CH], cs_c, sn_c)
        cb = rope_muls(psk[:, CH:2 * CH], cs_c, sn_c)
        for i in range(4):
            t = 4 * c + i
            vdst = v_sb[t][:].rearrange("p (h x) -> p h x", x=65)[:, :, 0:64]
            vsrc = psv[:, 256 * i:256 * i + 256].rearrange(
                "p (h x) -> p h x", x=64)
            nc.vector.tensor_copy(vdst, vsrc)
        rope_combine(*ca, k2a, lsl)
        rope_combine(*cb, k2b, lsl)

    def emit_scores_tile(c, t):
        """Scores + exp (+ causal mask) for kv tile t of chunk c.
        Returns the two bf16 expt tiles and the causal column offset."""
        ksl = slice(128 * t, 128 * t + 128)
        diag = (t // 4 == c)
        off = 128 * t - CH * c if diag else 0
        strip0 = ps.tile([128, 2 * CH], F32, tag="scP0")
        strip1 = ps.tile([128, 2 * CH], F32, tag="scP1")
        strips = [strip0, strip1]
        for h in range(HPC):
            k2 = (k2a, k2b)[h // 2]
            q2 = (q2a, q2b)[h // 2]
            hsl = slice(64 * (h % 2), 64 * (h % 2) + 64)
            pss = strips[h // 2][:, CH * (h % 2) + off:CH * (h % 2 + 1)]
            nc.tensor.matmul(pss, k2[hsl, ksl],
                             q2[hsl, CH * c + off:CH * (c + 1)],
                             start=True, stop=True,
                             tile_position=(64 * (h % 2), 0))
        expts = []
        for p in range(2):
            strip = strips[p]
            expt = epool.tile([128, 2 * CH], BF16, tag="expt")
            esrc = strip[:].rearrange("q (h x) -> q h x", x=CH)[:, :, off:]
            edst = expt[:].rearrange("q (h x) -> q h x", x=CH)[:, :, off:]
            nc.scalar.activation(edst, esrc, AF.Exp, scale=0.125)
            if diag:
                # triangular mask on the 128-wide diagonal block
                for hh in range(2):
                    blk = slice(CH * hh + off, CH * hh + off + 128)
                    nc.vector.tensor_mul(expt[:, blk], expt[:, blk], tri)
            expts.append(expt)
        return expts, off

    def emit_av_tile(c, t, av, expts, off):
        ntile = 4 * c + 4
        for p in range(2):
            for hh in range(2):
                h = 2 * p + hh
                nc.tensor.matmul(av[h][:, off:],
                                 v_sb[t][:, 65 * h:65 * h + 65],
                                 expts[p][:, CH * hh + off:CH * (hh + 1)],
                                 start=(t == 0), stop=(t == ntile - 1))

    def alloc_av():
        av = []
        for h in range(HPC):
            t_av = ps.tile([65, CH], F32, tag=f"av{h}", name="t_av")
            av.append(t_av)
        return av

    def emit_attention(c, av, t0=0, pre=()):
        """AV for prefetched tiles `pre`, then full tiles t0..ntile-1."""
        for t, (expts, off) in enumerate(pre):
            emit_av_tile(c, t, av, expts, off)
        for t in range(t0, 4 * c + 4):
            expts, off = emit_scores_tile(c, t)
            emit_av_tile(c, t, av, expts, off)

    def emit_norm(c, av):
        """Softmax normalization: ho = av * (1/denom) for chunk c.

        PE-free: reciprocals (DVE) -> partition broadcasts (Pool) -> scale
        (DVE), all recips first so the Pool broadcasts pipeline behind them.
        """
        qsl = slice(CH * c, CH * (c + 1))
        bcs = []
        for h in range(HPC):
            inv = ipool.tile([1, CH], F32R, tag=f"inv{h}")
            with nc.allow_low_precision(reason="tf32 softmax denom"):
                nc.vector.reciprocal(inv, av[h][64:65, :])
            bch = ipool.tile([64, CH], F32R, tag=f"bc{h}")
            nc.gpsimd.partition_broadcast(bch, inv)
            bcs.append(bch)
        for h in range(HPC):
            nc.vector.tensor_mul(
                ho[h // 2][64 * (h % 2):64 * (h % 2) + 64, qsl],
                av[h][0:64, :], bcs[h])

    def emit_outproj_group(c, g):
        """Half of chunk c's output projection (2 l-tiles x 2 column halves)
        on av0..av3 — each tag used once per group, j rounds interleaved,
        staging copies alternating DVE/ACT into one bf16 row-tile per lt."""
        tiles = []
        for i, lt in enumerate((4 * c + 2 * g, 4 * c + 2 * g + 1)):
            for oc in range(2):
                ps_o = ps.tile([128, 512], F32, tag=f"av{2 * i + oc}",
                               name="ps_o")
                tiles.append((ps_o, lt, oc))
        for j in range(2):
            for ps_o, lt, oc in tiles:
                osl = slice(512 * oc, 512 * oc + 512)
                nc.tensor.matmul(ps_o, ho[j][:, 128 * lt:128 * lt + 128],
                                 wo_sb[j][:, osl],
                                 start=(j == 0), stop=(j == 1))
        for i, lt in enumerate((4 * c + 2 * g, 4 * c + 2 * g + 1)):
            o_sb = opool.tile([128, DM], BF16, tag="o")
            nc.vector.tensor_copy(o_sb[:, 0:512], tiles[2 * i][0][:])
            nc.scalar.copy(o_sb[:, 512:DM], tiles[2 * i + 1][0][:])
            nc.sync.dma_start(out[128 * lt:128 * lt + 128, :], o_sb)

    # ---- software pipeline ----
    wv_sb.extend(_load_wv())
    emit_proj_q(0)
    emit_proj_kv(0)
    wo_sb.extend(_load_wo())
    emit_proj_q(1)
    emit_proj_kv(1)
    for c in range(2):
        av = alloc_av()
        emit_attention(c, av)
        # Q-projection PE work first so its RoPE products lead the DVE
        # queue; the normalize chain then hides under the K/V projections
        emit_proj_q(c + 2)
        emit_norm(c, av)
        emit_proj_kv(c + 2)
        emit_outproj_group(c, 0)
        emit_outproj_group(c, 1)
    # c = 2: no projection filler left — prefetch the first scores of
    # chunk 3 (deferring their AV matmuls) to keep PE fed during norm(2)
    av = alloc_av()
    emit_attention(2, av)
    pre = [emit_scores_tile(3, t) for t in range(4)]
    emit_norm(2, av)
    emit_outproj_group(2, 0)
    emit_outproj_group(2, 1)
    # c = 3
    av = alloc_av()
    emit_attention(3, av, t0=4, pre=pre)
    emit_norm(3, av)
    emit_outproj_group(3, 0)
    emit_outproj_group(3, 1)


def _build_nc():
    nc = bacc.Bacc("TRN2", target_bir_lowering=False, debug=False,
                   enable_asserts=False, num_devices=8)
    ins = {
        "xt": nc.dram_tensor("xt", [DM, L], BF16, kind="ExternalInput").ap(),
        "wq": nc.dram_tensor("wq", [DM, 512], BF16, kind="ExternalInput").ap(),
        "wv": nc.dram_tensor("wv", [DM, 256], BF16, kind="ExternalInput").ap(),
        "wo": nc.dram_tensor("wo", [256, DM], F32, kind="ExternalInput").ap(),
        "cs": nc.dram_tensor("cs", [128, L], F32, kind="ExternalInput").ap(),
        "sn": nc.dram_tensor("sn", [128, L], F32, kind="ExternalInput").ap(),
    }
    outs = {"out": nc.dram_tensor("out", [L, DM], BF16, kind="ExternalOutput").ap()}
    with tile.TileContext(nc) as tc:
        _attn_kernel(tc, outs, ins)
    nc.compile()
    return nc


def _host_shard(X, token_positions, Wqkv, Wout):
    """Build the 8 per-core input maps."""
    X = np.asarray(X, dtype=np.float32)
    Wqkv = np.asarray(Wqkv, dtype=np.float32)
    Wout = np.asarray(Wout, dtype=np.float32)
    pos = np.asarray(token_positions)

    # Per-head dim order: [e0..e15, o0..o15, e16..e31, o16..o31] so the RoPE
    # partner (even<->odd of the same freq) is p^16 within a 32-quadrant.
    def head_order(base):
        return ([base + 2 * k for k in range(16)] +
                [base + 2 * k + 1 for k in range(16)] +
                [base + 2 * k for k in range(16, 32)] +
                [base + 2 * k + 1 for k in range(16, 32)])

    # RoPE tables in float32 arithmetic to mirror the f32 reference
    k = np.arange(DK // 2, dtype=np.float32)
    inv_freq = (np.float32(1.0) /
                np.power(np.float32(THETA), (np.float32(2.0) * k) / np.float32(DK)))
    inv_freq = inv_freq.astype(np.float32)
    # freq index per partition within a 64-block: [0:16]=f0..15, [16:32]=f0..15,
    # [32:48]=f16..31, [48:64]=f16..31; sign +1 on top blocks, -1 on bot blocks
    fidx = np.concatenate([np.arange(16), np.arange(16),
                           np.arange(16, 32), np.arange(16, 32)])
    sgn = np.concatenate([np.ones(16), -np.ones(16),
                          np.ones(16), -np.ones(16)]).astype(np.float32)
    fidx = np.tile(fidx, 2)           # 128 partitions (2 heads per tile)
    sgn = np.tile(sgn, 2)
    ang = (pos.astype(np.float32)[:, None, :] *
           inv_freq[fidx][None, :, None]).astype(np.float32)   # [B, 128, L]
    cs_all = np.cos(ang).astype(np.float32)
    sn_all = (np.sin(ang) * sgn[None, :, None]).astype(np.float32)

    in_maps = []
    for core in range(8):
        b, g = divmod(core, HPC)
        heads = [HPC * g + hh for hh in range(HPC)]
        q_rows, k_rows = [], []
        for h in heads:
            q_rows += head_order(DK * h)
            k_rows += head_order(DM + DK * h)
        wq_c = np.ascontiguousarray(Wqkv[q_rows + k_rows, :].T
                                    .astype(ml_dtypes.bfloat16))
        v_rows = [2 * DM + DK * h + j for h in heads for j in range(DK)]
        wv_c = np.ascontiguousarray(Wqkv[v_rows, :].T.astype(ml_dtypes.bfloat16))
        wo_c = np.ascontiguousarray(Wout[:, 256 * g:256 * (g + 1)].T)
        in_maps.append({
            "xt": np.ascontiguousarray(X[b].T.astype(ml_dtypes.bfloat16)),
            "wq": wq_c,
            "wv": wv_c,
            "wo": wo_c,
            "cs": np.ascontiguousarray(cs_all[b]),
            "sn": np.ascontiguousarray(sn_all[b]),
        })
    return in_maps


def kernel(X, token_positions, Wqkv, Wout, _trace=False):
    if "nc" not in _cache:
        _cache["nc"] = _build_nc()
    nc = _cache["nc"]
    in_maps = _host_shard(X, token_positions, Wqkv, Wout)
    res = run_bass_kernel_spmd(nc, in_maps, list(range(8)), trace=_trace)
    _cache["last_results"] = res
    out = np.zeros((B, L, DM), dtype=np.float32)
    for core in range(8):
        out[core // HPC] += np.asarray(res.results[core]["out"],
                                       dtype=np.float32)
    return out
